# revision 8
# baseline (speedup 1.0000x reference)
"""BlockDiffusionDecoder (mBART-style 2-layer decoder + BD3LM self-attn mask)
on 8 Trainium2 NeuronCores.

Sharding: cores (2b, 2b+1) own batch element b (B=4 -> 8 cores).  Within a
pair, tensor-parallel over heads (8 of 16) and d_ff (2048 of 4096), with a
pair AllReduce after the o-projections and fc2.  The LM head is sharded over
vocab 8 ways (padded 32768 = 8 x 4096) after an AllGather of final hidden
states across the {even} / {odd} core groups.

Layouts: activations live in SBUF as [128 tokens, tile, feature]; transposed
copies ([feature-tile, token]) are built with PE transposes.  All matmuls run
in bf16 (full PE rate); residual stream / LN / softmax stats stay fp32.
Weights are shipped from host pre-tiled and pre-cast to bf16.
"""
import sys

if "/opt/trn_rl_repo" not in sys.path:
    sys.path.insert(0, "/opt/trn_rl_repo")

import contextlib

import ml_dtypes
import numpy as np

import concourse.bass as bass
import concourse.bacc as bacc
import concourse.tile as tile
from concourse import mybir
from concourse.bass_utils import run_bass_kernel_spmd
from concourse.masks import make_identity

P = 128
B, D, H, NL, DFF, V, S = 4, 1024, 16, 2, 4096, 32000, 128
T = 1024
HD = D // H          # 64
BLK = 4
VP = 32768           # padded vocab (32001 -> 8*4096)
VSH = VP // 8        # vocab shard per core
NT = T // P          # 8 token tiles
ND = D // P          # 8 feature tiles
EMB_SCALE = 32.0     # sqrt(D)
FMIN = float(np.finfo(np.float32).min)
BF = ml_dtypes.bfloat16

f32 = mybir.dt.float32
bf16 = mybir.dt.bfloat16
i32 = mybir.dt.int32
AF = mybir.ActivationFunctionType
ALU = mybir.AluOpType
AX = mybir.AxisListType


def _rhs_tile(w_t: np.ndarray, nchunk: int) -> np.ndarray:
    """[d_in, d_out] -> [n_chunks, 128, k_tiles, nchunk] bf16, so the DMA of
    one n-chunk is contiguous per partition (k-major, n-minor)."""
    d_in, d_out = w_t.shape
    kt = d_in // P
    nc_ = d_out // nchunk
    return np.ascontiguousarray(
        w_t.reshape(kt, P, nc_, nchunk).transpose(2, 1, 0, 3).astype(BF))


def host_prepare(inputs: dict, hsh: int, dsh: int, vsh: int):
    """Build per-core input maps. hsh: heads/core, dsh: d_ff/core."""
    tp = 16 // hsh
    ids = np.asarray(inputs["input_ids"])
    enc = np.asarray(inputs["enc_hidden"], dtype=np.float32)
    emask = np.asarray(inputs["enc_mask"])
    emb = np.ascontiguousarray(np.asarray(inputs["embed_tokens"], np.float32))
    pos = np.ascontiguousarray(np.asarray(inputs["pos_embed"], np.float32))
    attn_w = np.asarray(inputs["attn_w"], np.float32)
    attn_b = np.asarray(inputs["attn_b"], np.float32)
    ln_w = np.asarray(inputs["ln_w"], np.float32)
    ln_b = np.asarray(inputs["ln_b"], np.float32)
    fc1_w = np.asarray(inputs["fc1_w"], np.float32)
    fc1_b = np.asarray(inputs["fc1_b"], np.float32)
    fc2_w = np.asarray(inputs["fc2_w"], np.float32)
    fc2_b = np.asarray(inputs["fc2_b"], np.float32)
    lm_w = np.asarray(inputs["lm_head_w"], np.float32)

    lm_pad = np.zeros((VP, D), np.float32)
    lm_pad[: V + 1] = lm_w
    lm_t = lm_pad.T  # [D, VP]

    n_cores = 8 if tp == 2 else 1
    maps = []
    for c in range(n_cores):
        b_ = c // tp
        j = c % tp
        hs = slice(j * hsh * HD, (j + 1) * hsh * HD)
        ds_ = slice(j * dsh, (j + 1) * dsh)
        vs_ = slice(c * vsh, (c + 1) * vsh) if tp == 2 else slice(0, vsh)
        m = {
            "ids": ids[b_].reshape(T, 1).astype(np.int32),
            "emb": emb,
            "pos": pos,
            "encT": np.ascontiguousarray(enc[b_].T.astype(BF)),   # [D, S]
            "cmask": ((1.0 - emask[b_].astype(np.float32)) * FMIN)
            .reshape(1, S),
            "lnemb": np.stack([np.asarray(inputs["ln_emb_s"], np.float32),
                               np.asarray(inputs["ln_emb_b"], np.float32)]),
            "lnfin": np.stack([np.asarray(inputs["final_ln_s"], np.float32),
                               np.asarray(inputs["final_ln_b"], np.float32)]),
            "lnw": ln_w, "lnb": ln_b,
            "wlm": np.ascontiguousarray(
                lm_t[:, vs_].reshape(ND, P, vsh).transpose(1, 0, 2)
                .astype(BF)),
        }
        for l in range(NL):
            for a, tag in ((0, "s"), (1, "c")):
                wq, wk, wv, wo = attn_w[l, a]
                bq, bk, bv, bo = attn_b[l, a]
                m[f"wq{tag}{l}"] = _rhs_tile(wq.T[:, hs], P)
                m[f"wk{tag}{l}"] = _rhs_tile(wk.T[:, hs], P)
                m[f"wv{tag}{l}"] = _rhs_tile(wv.T[:, hs], hsh * HD)
                m[f"wo{tag}{l}"] = _rhs_tile(wo.T[hs, :], D // 2)
                mh = hsh * HD // P
                m[f"bq{tag}{l}"] = np.ascontiguousarray(
                    bq[hs].reshape(mh, P).T)
                m[f"bk{tag}{l}"] = np.ascontiguousarray(
                    bk[hs].reshape(mh, P).T)
                m[f"bv{tag}{l}"] = bv[hs].reshape(1, hsh * HD).copy()
                m[f"bo{tag}{l}"] = (bo / tp).reshape(1, D).copy()
            m[f"wf1{l}"] = _rhs_tile(fc1_w[l].T[:, ds_], 512)
            m[f"bf1{l}"] = fc1_b[l][ds_].reshape(1, dsh).copy()
            m[f"wf2{l}"] = np.ascontiguousarray(
                fc2_w[l].T[ds_, :].reshape(dsh // 512, 4, P, D)
                .transpose(0, 2, 1, 3).astype(BF))  # [dffc, p, kk, D]
            m[f"bf2{l}"] = (fc2_b[l] / tp).reshape(1, D).copy()
        maps.append(m)
    return maps


def _mask_consts():
    i = np.arange(P)
    diag = np.where((i[:, None] // BLK) == (i[None, :] // BLK), 0.0, FMIN)
    tri_s = np.where((i[:, None] // BLK) > (i[None, :] // BLK), 0.0, FMIN)
    tri_i = np.where((i[:, None] // BLK) >= (i[None, :] // BLK), 0.0, FMIN)
    return (diag.astype(np.float32), tri_s.astype(np.float32),
            tri_i.astype(np.float32))


def build_nc(hsh=8, dsh=2048, vsh=VSH, nb_lm=4, collectives=True,
             gelu=AF.Gelu_apprx_tanh):
    tp = 16 // hsh
    MH = hsh * HD // P        # d_out tiles for q/k/v shard
    KO = MH                   # k-tiles for o-proj lhs
    NDC = dsh // 512          # dff chunks
    nc = bacc.Bacc(num_devices=8 if collectives else None, trn_type="TRN2")

    ids_d = nc.dram_tensor("ids", [T, 1], i32, kind="ExternalInput")
    emb_d = nc.dram_tensor("emb", [V + 1, D], f32, kind="ExternalInput")
    pos_d = nc.dram_tensor("pos", [T, D], f32, kind="ExternalInput")
    encT_d = nc.dram_tensor("encT", [D, S], bf16, kind="ExternalInput")
    cmask_d = nc.dram_tensor("cmask", [1, S], f32, kind="ExternalInput")
    lnemb_d = nc.dram_tensor("lnemb", [2, D], f32, kind="ExternalInput")
    lnfin_d = nc.dram_tensor("lnfin", [2, D], f32, kind="ExternalInput")
    lnw_d = nc.dram_tensor("lnw", [NL, 3, D], f32, kind="ExternalInput")
    lnb_d = nc.dram_tensor("lnb", [NL, 3, D], f32, kind="ExternalInput")
    wlm_d = nc.dram_tensor("wlm", [P, ND, vsh], bf16, kind="ExternalInput")
    wd, bd = {}, {}
    for l in range(NL):
        for tg in ("s", "c"):
            wd[f"wq{tg}{l}"] = nc.dram_tensor(
                f"wq{tg}{l}", [MH, P, ND, P], bf16, kind="ExternalInput")
            wd[f"wk{tg}{l}"] = nc.dram_tensor(
                f"wk{tg}{l}", [MH, P, ND, P], bf16, kind="ExternalInput")
            wd[f"wv{tg}{l}"] = nc.dram_tensor(
                f"wv{tg}{l}", [1, P, ND, hsh * HD], bf16,
                kind="ExternalInput")
            wd[f"wo{tg}{l}"] = nc.dram_tensor(
                f"wo{tg}{l}", [2, P, KO, D // 2], bf16, kind="ExternalInput")
            bd[f"bq{tg}{l}"] = nc.dram_tensor(
                f"bq{tg}{l}", [P, MH], f32, kind="ExternalInput")
            bd[f"bk{tg}{l}"] = nc.dram_tensor(
                f"bk{tg}{l}", [P, MH], f32, kind="ExternalInput")
            bd[f"bv{tg}{l}"] = nc.dram_tensor(
                f"bv{tg}{l}", [1, hsh * HD], f32, kind="ExternalInput")
            bd[f"bo{tg}{l}"] = nc.dram_tensor(
                f"bo{tg}{l}", [1, D], f32, kind="ExternalInput")
        wd[f"wf1{l}"] = nc.dram_tensor(
            f"wf1{l}", [NDC, P, ND, 512], bf16, kind="ExternalInput")
        bd[f"bf1{l}"] = nc.dram_tensor(
            f"bf1{l}", [1, dsh], f32, kind="ExternalInput")
        wd[f"wf2{l}"] = nc.dram_tensor(
            f"wf2{l}", [NDC, P, 4, D], bf16, kind="ExternalInput")
        bd[f"bf2{l}"] = nc.dram_tensor(
            f"bf2{l}", [1, D], f32, kind="ExternalInput")
    out_d = nc.dram_tensor("out", [nb_lm * T, vsh], f32,
                           kind="ExternalOutput")

    mdiag_np, mtris_np, mtrii_np = _mask_consts()
    mdiag_d = nc.inline_tensor(mdiag_np, "mdiag")
    mtris_d = nc.inline_tensor(mtris_np, "mtris")
    mtrii_d = nc.inline_tensor(mtrii_np, "mtrii")

    PAIRS = [[0, 1], [2, 3], [4, 5], [6, 7]]
    EVENODD = [[0, 2, 4, 6], [1, 3, 5, 7]]

    def bcast(ap_1d, p=P):
        return bass.AP(tensor=ap_1d.tensor, offset=ap_1d.offset,
                       ap=[[0, p]] + list(ap_1d.ap))

    with tile.TileContext(nc) as tc:
        gctx = contextlib.ExitStack()
        with gctx:
            consts = gctx.enter_context(tc.tile_pool(name="consts", bufs=1))
            small = gctx.enter_context(tc.tile_pool(name="small", bufs=4))
            sp = gctx.enter_context(tc.tile_pool(name="sp", bufs=2))
            xp = gctx.enter_context(tc.tile_pool(name="xp", bufs=3))
            xb = gctx.enter_context(tc.tile_pool(name="xb", bufs=2))
            dram = gctx.enter_context(
                tc.tile_pool(name="dram", bufs=1, space="DRAM"))
            ps_a = gctx.enter_context(
                tc.tile_pool(name="ps_a", bufs=4, space="PSUM"))
            ps_av = gctx.enter_context(
                tc.tile_pool(name="ps_av", bufs=2, space="PSUM"))
            ps_tr = gctx.enter_context(
                tc.tile_pool(name="ps_tr", bufs=2, space="PSUM"))

            ident = consts.tile([P, P], bf16)
            make_identity(nc, ident[:])
            eps_t = consts.tile([P, 1], f32)
            nc.vector.memset(eps_t[:], 1e-5)
            mdiag = consts.tile([P, P], f32)
            nc.sync.dma_start(out=mdiag[:], in_=mdiag_d[:])
            mtris = consts.tile([P, P], f32)
            nc.sync.dma_start(out=mtris[:], in_=mtris_d[:])
            mtrii = consts.tile([P, P], f32)
            nc.sync.dma_start(out=mtrii[:], in_=mtrii_d[:])
            cmask_b = consts.tile([P, S], f32)
            nc.sync.dma_start(out=cmask_b[:], in_=bcast(cmask_d[0]))

            cc_in = dram.tile([T, D], f32)
            cc_out = (dram.tile([T, D], f32, name="cc_out")
                      if collectives else cc_in)
            ccf_in = dram.tile([T, D], f32)
            ccf_out = (dram.tile([nb_lm, T, D], f32, name="ccf_out")
                       if collectives else ccf_in)

            def ln_tile(src_ap, dst_ap, s_b, b_b):
                st = small.tile([P, 2, 6], f32, name="lnstats")
                nc.vector.bn_stats(out=st[:, 0, :], in_=src_ap[:, 0:512])
                nc.vector.bn_stats(out=st[:, 1, :], in_=src_ap[:, 512:1024])
                mv = small.tile([P, 2], f32, name="lnmv")
                nc.vector.bn_aggr(out=mv[:], in_=st[:])
                rstd = small.tile([P, 1], f32, name="lnrstd")
                nc.scalar.activation(out=rstd[:], in_=mv[:, 1:2],
                                     func=AF.Sqrt, bias=eps_t[:])
                nc.vector.reciprocal(out=rstd[:], in_=rstd[:])
                tmp = xp.tile([P, D], f32, name="xrow")
                nc.vector.tensor_scalar(out=tmp[:], in0=src_ap,
                                        scalar1=mv[:, 0:1], scalar2=rstd[:],
                                        op0=ALU.subtract, op1=ALU.mult)
                nc.vector.tensor_tensor(out=tmp[:], in0=tmp[:], in1=s_b,
                                        op=ALU.mult)
                nc.vector.tensor_tensor(out=dst_ap, in0=tmp[:], in1=b_b,
                                        op=ALU.add)

            # ---------------- stack phase ----------------
            sctx = contextlib.ExitStack()
            with sctx:
                pers = sctx.enter_context(tc.tile_pool(name="pers", bufs=1))
                wp = sctx.enter_context(tc.tile_pool(name="wp", bufs=2))
                lnp = sctx.enter_context(tc.tile_pool(name="lnp", bufs=1))

                h = pers.tile([P, NT, D], f32)
                encT = pers.tile([P, ND, S], bf16)
                nc.sync.dma_start(
                    out=encT[:],
                    in_=encT_d.rearrange("(k p) s -> p k s", p=P))

                def ln_consts(s_src, b_src):
                    s_b = lnp.tile([P, D], f32, name="ln_s")
                    nc.sync.dma_start(out=s_b[:], in_=bcast(s_src))
                    b_b = lnp.tile([P, D], f32, name="ln_b")
                    nc.sync.dma_start(out=b_b[:], in_=bcast(b_src))
                    return s_b, b_b

                def ln_to_xT(dst_xT, s_b, b_b):
                    """x = LN(h) (bf16) then xT[:, k, tt*128:] = T(x)."""
                    for tt in range(NT):
                        xt_ = xb.tile([P, D], bf16, name="xbrow")
                        ln_tile(h[:, tt, :], xt_[:], s_b[:], b_b[:])
                        for k in range(ND):
                            tp_ = ps_tr.tile([P, P], bf16, name="trps")
                            nc.tensor.transpose(
                                tp_[:], xt_[:, k * P:(k + 1) * P], ident[:])
                            nc.scalar.activation(
                                out=dst_xT[:, k, tt * P:(tt + 1) * P],
                                in_=tp_[:], func=AF.Copy)

                def proj_qk(dst, w_key, b_sb, scale, src_xT):
                    """dst[pd, m, t] = (xT.T @ W)^T with bias (+opt scale)."""
                    for m in range(MH):
                        wch = wp.tile([P, ND, P], bf16, name="wch")
                        nc.sync.dma_start(out=wch[:], in_=wd[w_key][m])
                        for half in range(2):
                            psq = ps_a.tile([P, 512], f32, name="psq")
                            for k in range(ND):
                                nc.tensor.matmul(
                                    out=psq[:],
                                    lhsT=wch[:, k, :],
                                    rhs=src_xT[:, k,
                                               half * 512:(half + 1) * 512],
                                    start=(k == 0), stop=(k == ND - 1))
                            if scale is None:
                                nc.vector.tensor_scalar(
                                    out=dst[:, m, half * 512:(half + 1) * 512],
                                    in0=psq[:], scalar1=b_sb[:, m:m + 1],
                                    scalar2=None, op0=ALU.add)
                            else:
                                nc.vector.tensor_scalar(
                                    out=dst[:, m, half * 512:(half + 1) * 512],
                                    in0=psq[:], scalar1=b_sb[:, m:m + 1],
                                    scalar2=scale, op0=ALU.add, op1=ALU.mult)

                def oproj_ar_update(src_oT, wo_key, bo_key):
                    """o-proj partial -> cc_in -> AllReduce -> h update."""
                    bo_b = lnp.tile([P, D], f32, name="bo_b")
                    nc.sync.dma_start(out=bo_b[:], in_=bcast(bd[bo_key][0]))
                    for half in range(2):
                        wch = wp.tile([P, KO, 512], bf16, name="wch")
                        nc.sync.dma_start(out=wch[:], in_=wd[wo_key][half])
                        for tt in range(NT):
                            pso = ps_a.tile([P, 512], f32, name="psq")
                            for k in range(KO):
                                nc.tensor.matmul(
                                    out=pso[:],
                                    lhsT=src_oT[:, k, tt * P:(tt + 1) * P],
                                    rhs=wch[:, k, :],
                                    start=(k == 0), stop=(k == KO - 1))
                            ev = xp.tile([P, 512], f32, name="ev512")
                            if tt % 2 == 0:
                                nc.scalar.activation(out=ev[:], in_=pso[:],
                                                     func=AF.Copy)
                            else:
                                nc.vector.tensor_copy(out=ev[:], in_=pso[:])
                            nc.sync.dma_start(
                                out=cc_in[tt * P:(tt + 1) * P,
                                          half * 512:(half + 1) * 512],
                                in_=ev[:])
                    if collectives:
                        nc.gpsimd.collective_compute(
                            "AllReduce", ALU.add, replica_groups=PAIRS,
                            ins=[cc_in[:]], outs=[cc_out[:]])
                    for tt in range(NT):
                        dtile = xp.tile([P, D], f32, name="xrow")
                        nc.sync.dma_start(
                            out=dtile[:], in_=cc_out[tt * P:(tt + 1) * P, :])
                        nc.vector.tensor_tensor(out=dtile[:], in0=dtile[:],
                                                in1=bo_b[:], op=ALU.add)
                        nc.vector.tensor_tensor(out=h[:, tt, :],
                                                in0=h[:, tt, :],
                                                in1=dtile[:], op=ALU.add)

                # ---- embed + emb LN ----
                lnes, lneb = ln_consts(lnemb_d[0], lnemb_d[1])
                for tt in range(NT):
                    idt = small.tile([P, 1], i32, name="idt")
                    nc.sync.dma_start(out=idt[:],
                                      in_=ids_d[tt * P:(tt + 1) * P])
                    g = xp.tile([P, D], f32, name="xrow")
                    nc.gpsimd.indirect_dma_start(
                        out=g[:], out_offset=None, in_=emb_d[:],
                        in_offset=bass.IndirectOffsetOnAxis(
                            ap=idt[:, :1], axis=0))
                    pt = xp.tile([P, D], f32, name="xrow")
                    nc.sync.dma_start(out=pt[:],
                                      in_=pos_d[tt * P:(tt + 1) * P])
                    nc.vector.tensor_scalar(out=g[:], in0=g[:],
                                            scalar1=EMB_SCALE, scalar2=None,
                                            op0=ALU.mult)
                    nc.vector.tensor_tensor(out=h[:, tt, :], in0=g[:],
                                            in1=pt[:], op=ALU.add)
                    ln_tile(h[:, tt, :], h[:, tt, :], lnes[:], lneb[:])

                for l in range(NL):
                    # ======== self attention ========
                    lns, lnbb = ln_consts(lnw_d[l, 0], lnb_d[l, 0])
                    xT = pers.tile([P, ND, T], bf16, name="xT", tag="xT")
                    ln_to_xT(xT, lns, lnbb)

                    qT = pers.tile([P, MH, T], bf16, name="qT", tag="qT")
                    kT = pers.tile([P, MH, T], bf16, name="kT", tag="kT")
                    vv = pers.tile([P, NT, hsh * HD], bf16, name="vv",
                                   tag="vv")
                    bq_sb = small.tile([P, MH], f32, name="bq")
                    nc.sync.dma_start(out=bq_sb[:], in_=bd[f"bqs{l}"][:])
                    bk_sb = small.tile([P, MH], f32, name="bk")
                    nc.sync.dma_start(out=bk_sb[:], in_=bd[f"bks{l}"][:])
                    bv_b = lnp.tile([P, hsh * HD], f32, name="bv_b")
                    nc.sync.dma_start(out=bv_b[:],
                                      in_=bcast(bd[f"bvs{l}"][0]))

                    proj_qk(qT, f"wqs{l}", bq_sb, 0.125, xT)
                    proj_qk(kT, f"wks{l}", bk_sb, None, xT)
                    wch_v = wp.tile([P, ND, hsh * HD], bf16, name="wch")
                    nc.sync.dma_start(out=wch_v[:], in_=wd[f"wvs{l}"][0])
                    for tt in range(NT):
                        psv = ps_a.tile([P, 512], f32, name="psq")
                        for k in range(ND):
                            nc.tensor.matmul(
                                out=psv[:],
                                lhsT=xT[:, k, tt * P:(tt + 1) * P],
                                rhs=wch_v[:, k, :],
                                start=(k == 0), stop=(k == ND - 1))
                        nc.vector.tensor_tensor(out=vv[:, tt, :], in0=psv[:],
                                                in1=bv_b[:], op=ALU.add)

                    # scores + softmax + AV per head
                    oT = pers.tile([P, KO, T], bf16, name="oT", tag="oT")
                    for hl in range(hsh):
                        prow = slice((hl % 2) * 64, (hl % 2) * 64 + 64)
                        mq = hl // 2
                        ptile = sp.tile([P, 5, T], bf16, name="ptile")
                        for g in range(NT):
                            width = (g + 2) * P if g < 4 else (g - 3) * P
                            ssb = sp.tile([P, 640], f32, name="srow")
                            qst = qT[prow, mq, g * P:(g + 1) * P]
                            if g < 4:
                                psd = ps_a.tile([P, 512], f32, name="psq")
                                nc.tensor.matmul(
                                    out=psd[:, 0:P], lhsT=qst,
                                    rhs=kT[prow, mq, g * P:(g + 1) * P],
                                    start=True, stop=True)
                                pss = ps_a.tile([P, 512], f32, name="psq")
                                nc.tensor.matmul(
                                    out=pss[:, 0:width - P], lhsT=qst,
                                    rhs=kT[prow, mq, 512:512 + width - P],
                                    start=True, stop=True)
                                nc.vector.tensor_tensor(
                                    out=ssb[:, 0:P], in0=psd[:, 0:P],
                                    in1=mdiag[:], op=ALU.add)
                                if g > 0:
                                    nc.scalar.activation(
                                        out=ssb[:, P:width - P],
                                        in_=pss[:, 0:width - 2 * P],
                                        func=AF.Copy)
                                nc.vector.tensor_tensor(
                                    out=ssb[:, width - P:width],
                                    in0=pss[:, width - 2 * P:width - P],
                                    in1=mtris[:], op=ALU.add)
                            else:
                                pss = ps_a.tile([P, 512], f32, name="psq")
                                nc.tensor.matmul(
                                    out=pss[:, 0:width], lhsT=qst,
                                    rhs=kT[prow, mq, 512:512 + width],
                                    start=True, stop=True)
                                if width > P:
                                    nc.scalar.activation(
                                        out=ssb[:, 0:width - P],
                                        in_=pss[:, 0:width - P], func=AF.Copy)
                                nc.vector.tensor_tensor(
                                    out=ssb[:, width - P:width],
                                    in0=pss[:, width - P:width],
                                    in1=mtrii[:], op=ALU.add)
                            negmax = small.tile([P, 1], f32, name="negmax")
                            nc.vector.tensor_reduce(
                                out=negmax[:], in_=ssb[:, 0:width],
                                axis=AX.X, op=ALU.max, negate=True)
                            probs = sp.tile([P, 640], bf16, name="brow")
                            sums = small.tile([P, 1], f32, name="sums")
                            nc.scalar.activation(
                                out=probs[:, 0:width], in_=ssb[:, 0:width],
                                func=AF.Exp, bias=negmax[:],
                                accum_out=sums[:])
                            recip = small.tile([P, 1], f32, name="recip")
                            nc.vector.reciprocal(out=recip[:], in_=sums[:])
                            nc.vector.tensor_scalar(
                                out=probs[:, 0:width], in0=probs[:, 0:width],
                                scalar1=recip[:], scalar2=None, op0=ALU.mult)
                            if g < 4:
                                chunks = [(4, 0)] + [(mm, (mm + 1) * P)
                                                     for mm in range(g + 1)]
                            else:
                                chunks = [(mm, mm * P)
                                          for mm in range(g - 3)]
                            for slot, coff in chunks:
                                tpp = ps_tr.tile([P, P], bf16, name="trps")
                                nc.tensor.transpose(
                                    tpp[:], probs[:, coff:coff + P],
                                    ident[:])
                                nc.scalar.activation(
                                    out=ptile[:, slot, g * P:(g + 1) * P],
                                    in_=tpp[:], func=AF.Copy)
                        pav_lo = ps_av.tile([64, 512], f32, name="pav")
                        pav_hi = ps_av.tile([64, 512], f32, name="pav")
                        for mm in range(4):
                            nc.tensor.matmul(
                                out=pav_lo[:, mm * P:512],
                                lhsT=vv[:, 4 + mm, hl * HD:(hl + 1) * HD],
                                rhs=ptile[:, mm, mm * P:512],
                                start=(mm == 0), stop=False)
                            nc.tensor.matmul(
                                out=pav_hi[:, mm * P:512],
                                lhsT=vv[:, 4 + mm, hl * HD:(hl + 1) * HD],
                                rhs=ptile[:, mm, 512 + mm * P:T],
                                start=(mm == 0), stop=(mm == 3))
                        for g in range(4):
                            nc.tensor.matmul(
                                out=pav_lo[:, g * P:(g + 1) * P],
                                lhsT=vv[:, g, hl * HD:(hl + 1) * HD],
                                rhs=ptile[:, 4, g * P:(g + 1) * P],
                                start=False, stop=(g == 3))
                        nc.scalar.activation(out=oT[prow, mq, 0:512],
                                             in_=pav_lo[:], func=AF.Copy)
                        nc.scalar.activation(out=oT[prow, mq, 512:T],
                                             in_=pav_hi[:], func=AF.Copy)
                    oproj_ar_update(oT, f"wos{l}", f"bos{l}")

                    # ======== cross attention ========
                    lns2, lnb2 = ln_consts(lnw_d[l, 1], lnb_d[l, 1])
                    x2T = pers.tile([P, ND, T], bf16, name="x2T", tag="xT")
                    ln_to_xT(x2T, lns2, lnb2)
                    q2T = pers.tile([P, MH, T], bf16, name="q2T", tag="qT")
                    bq2_sb = small.tile([P, MH], f32, name="bq")
                    nc.sync.dma_start(out=bq2_sb[:], in_=bd[f"bqc{l}"][:])
                    bk2_sb = small.tile([P, MH], f32, name="bk")
                    nc.sync.dma_start(out=bk2_sb[:], in_=bd[f"bkc{l}"][:])
                    bv2_b = lnp.tile([P, hsh * HD], f32, name="bv_b")
                    nc.sync.dma_start(out=bv2_b[:],
                                      in_=bcast(bd[f"bvc{l}"][0]))
                    proj_qk(q2T, f"wqc{l}", bq2_sb, 0.125, x2T)

                    wch_k = wp.tile([P, ND, MH, P], bf16, name="wch")
                    for m in range(MH):
                        nc.sync.dma_start(out=wch_k[:, :, m, :],
                                          in_=wd[f"wkc{l}"][m])
                    kenc_raw = sp.tile([S, hsh * HD], bf16, name="kenc", bufs=1)
                    psk = ps_a.tile([P, 512], f32, name="psq")
                    for k in range(ND):
                        nc.tensor.matmul(out=psk[:], lhsT=encT[:, k, :],
                                         rhs=wch_k[:, k, :, :],
                                         start=(k == 0), stop=(k == ND - 1))
                    nc.scalar.activation(out=kenc_raw[:], in_=psk[:],
                                         func=AF.Copy)
                    kTe = sp.tile([P, MH, S], bf16, name="kTe", bufs=1)
                    for m in range(MH):
                        tpk = ps_tr.tile([P, P], bf16, name="trps")
                        nc.tensor.transpose(
                            tpk[:], kenc_raw[:, m * P:(m + 1) * P], ident[:])
                        nc.vector.tensor_scalar(
                            out=kTe[:, m, :], in0=tpk[:],
                            scalar1=bk2_sb[:, m:m + 1], scalar2=None,
                            op0=ALU.add)
                    wch_v2 = wp.tile([P, ND, hsh * HD], bf16, name="wch")
                    nc.sync.dma_start(out=wch_v2[:], in_=wd[f"wvc{l}"][0])
                    venc = sp.tile([S, hsh * HD], bf16, name="venc", bufs=1)
                    psv2 = ps_a.tile([P, 512], f32, name="psq")
                    for k in range(ND):
                        nc.tensor.matmul(out=psv2[:], lhsT=encT[:, k, :],
                                         rhs=wch_v2[:, k, :],
                                         start=(k == 0), stop=(k == ND - 1))
                    nc.vector.tensor_tensor(out=venc[:], in0=psv2[:],
                                            in1=bv2_b[:], op=ALU.add)

                    o2T = pers.tile([P, KO, T], bf16, name="o2T", tag="oT")
                    for hl in range(hsh):
                        prow = slice((hl % 2) * 64, (hl % 2) * 64 + 64)
                        mq = hl // 2
                        p2tile = sp.tile([S, T], bf16, name="ptile")
                        for tt in range(NT):
                            ps2 = ps_a.tile([P, 512], f32, name="psq")
                            nc.tensor.matmul(
                                out=ps2[:, 0:S],
                                lhsT=q2T[prow, mq, tt * P:(tt + 1) * P],
                                rhs=kTe[prow, mq, :], start=True, stop=True)
                            s2 = sp.tile([P, S], f32, name="srow")
                            nc.vector.tensor_tensor(out=s2[:],
                                                    in0=ps2[:, 0:S],
                                                    in1=cmask_b[:],
                                                    op=ALU.add)
                            negmax = small.tile([P, 1], f32, name="negmax")
                            nc.vector.tensor_reduce(
                                out=negmax[:], in_=s2[:], axis=AX.X,
                                op=ALU.max, negate=True)
                            probs2 = sp.tile([P, S], bf16, name="brow")
                            sums = small.tile([P, 1], f32, name="sums")
                            nc.scalar.activation(
                                out=probs2[:], in_=s2[:], func=AF.Exp,
                                bias=negmax[:], accum_out=sums[:])
                            recip = small.tile([P, 1], f32, name="recip")
                            nc.vector.reciprocal(out=recip[:], in_=sums[:])
                            nc.vector.tensor_scalar(
                                out=probs2[:], in0=probs2[:],
                                scalar1=recip[:], scalar2=None, op0=ALU.mult)
                            tpp = ps_tr.tile([P, P], bf16, name="trps")
                            nc.tensor.transpose(tpp[:], probs2[:], ident[:])
                            nc.scalar.activation(
                                out=p2tile[:, tt * P:(tt + 1) * P],
                                in_=tpp[:], func=AF.Copy)
                        for half in range(2):
                            pav2 = ps_av.tile([64, 512], f32, name="pav")
                            nc.tensor.matmul(
                                out=pav2[:],
                                lhsT=venc[:, hl * HD:(hl + 1) * HD],
                                rhs=p2tile[:, half * 512:(half + 1) * 512],
                                start=True, stop=True)
                            nc.scalar.activation(
                                out=o2T[prow, mq, half * 512:(half + 1) * 512],
                                in_=pav2[:], func=AF.Copy)
                    oproj_ar_update(o2T, f"woc{l}", f"boc{l}")

                    # ======== FFN ========
                    lns3, lnb3 = ln_consts(lnw_d[l, 2], lnb_d[l, 2])
                    x3T = pers.tile([P, ND, T], bf16, name="x3T", tag="xT")
                    ln_to_xT(x3T, lns3, lnb3)
                    bf1_b = lnp.tile([P, dsh], f32, name="bf1_b")
                    nc.sync.dma_start(out=bf1_b[:],
                                      in_=bcast(bd[f"bf1{l}"][0]))
                    bf2_b = lnp.tile([P, D], f32, name="bo_b")
                    nc.sync.dma_start(out=bf2_b[:],
                                      in_=bcast(bd[f"bf2{l}"][0]))
                    acc = pers.tile([P, NT, D], f32, name="acc", tag="oT")
                    for dffc in range(NDC):
                        f1c = wp.tile([P, ND, 512], bf16, name="wch")
                        nc.sync.dma_start(out=f1c[:],
                                          in_=wd[f"wf1{l}"][dffc])
                        f2c = wp.tile([P, 4, D], bf16, name="wch")
                        nc.sync.dma_start(out=f2c[:],
                                          in_=wd[f"wf2{l}"][dffc])
                        for tt in range(NT):
                            psf = ps_a.tile([P, 512], f32, name="psq")
                            for k in range(ND):
                                nc.tensor.matmul(
                                    out=psf[:],
                                    lhsT=x3T[:, k, tt * P:(tt + 1) * P],
                                    rhs=f1c[:, k, :],
                                    start=(k == 0), stop=(k == ND - 1))
                            gact = xb.tile([P, 512], bf16, name="gact")
                            nc.vector.tensor_tensor(
                                out=gact[:], in0=psf[:],
                                in1=bf1_b[:, dffc * 512:(dffc + 1) * 512],
                                op=ALU.add)
                            nc.scalar.activation(out=gact[:], in_=gact[:],
                                                 func=gelu)
                            gT = xb.tile([P, 4, P], bf16, name="gT")
                            for kk in range(4):
                                tpg = ps_tr.tile([P, P], bf16, name="trps")
                                nc.tensor.transpose(
                                    tpg[:], gact[:, kk * P:(kk + 1) * P],
                                    ident[:])
                                nc.scalar.activation(out=gT[:, kk, :],
                                                     in_=tpg[:],
                                                     func=AF.Copy)
                            for n2 in range(2):
                                psf2 = ps_a.tile([P, 512], f32, name="psq")
                                for kk in range(4):
                                    nc.tensor.matmul(
                                        out=psf2[:], lhsT=gT[:, kk, :],
                                        rhs=f2c[:, kk,
                                                n2 * 512:(n2 + 1) * 512],
                                        start=(kk == 0), stop=(kk == 3))
                                if dffc == 0:
                                    nc.vector.tensor_copy(
                                        out=acc[:, tt,
                                                n2 * 512:(n2 + 1) * 512],
                                        in_=psf2[:])
                                else:
                                    nc.vector.tensor_tensor(
                                        out=acc[:, tt,
                                                n2 * 512:(n2 + 1) * 512],
                                        in0=acc[:, tt,
                                                n2 * 512:(n2 + 1) * 512],
                                        in1=psf2[:], op=ALU.add)
                    for tt in range(NT):
                        nc.sync.dma_start(out=cc_in[tt * P:(tt + 1) * P, :],
                                          in_=acc[:, tt, :])
                    if collectives:
                        nc.gpsimd.collective_compute(
                            "AllReduce", ALU.add, replica_groups=PAIRS,
                            ins=[cc_in[:]], outs=[cc_out[:]])
                    for tt in range(NT):
                        dtile = xp.tile([P, D], f32, name="xrow")
                        nc.sync.dma_start(
                            out=dtile[:], in_=cc_out[tt * P:(tt + 1) * P, :])
                        nc.vector.tensor_tensor(out=dtile[:], in0=dtile[:],
                                                in1=bf2_b[:], op=ALU.add)
                        nc.vector.tensor_tensor(out=h[:, tt, :],
                                                in0=h[:, tt, :],
                                                in1=dtile[:], op=ALU.add)

                # ---- final LN -> ccf ----
                lnfs, lnfb = ln_consts(lnfin_d[0], lnfin_d[1])
                for tt in range(NT):
                    hf = xp.tile([P, D], f32, name="xrow")
                    ln_tile(h[:, tt, :], hf[:], lnfs[:], lnfb[:])
                    nc.sync.dma_start(out=ccf_in[tt * P:(tt + 1) * P, :],
                                      in_=hf[:])
            # stack pools closed here

            if collectives:
                nc.gpsimd.collective_compute(
                    "AllGather", ALU.bypass, replica_groups=EVENODD,
                    ins=[ccf_in[:]], outs=[ccf_out[:]])

            # ---------------- LM head ----------------
            lctx = contextlib.ExitStack()
            with lctx:
                lmp = lctx.enter_context(tc.tile_pool(name="lmp", bufs=1))
                lmt = lctx.enter_context(tc.tile_pool(name="lmt", bufs=2))
                wlm_sb = lmp.tile([P, ND, vsh], bf16)
                for k in range(ND):
                    nc.sync.dma_start(out=wlm_sb[:, k, :], in_=wlm_d[:, k, :])
                for bt in range(nb_lm):
                    for tt in range(NT):
                        hft = xb.tile([P, D], bf16, name="xbrow")
                        src = (ccf_out[bt, tt * P:(tt + 1) * P, :]
                               if collectives
                               else ccf_in[tt * P:(tt + 1) * P, :])
                        nc.gpsimd.dma_start(out=hft[:], in_=src)
                        hfT = lmt.tile([P, ND, P], bf16, name="hfT")
                        for k in range(ND):
                            tph = ps_tr.tile([P, P], bf16, name="trps")
                            nc.tensor.transpose(
                                tph[:], hft[:, k * P:(k + 1) * P], ident[:])
                            nc.scalar.activation(out=hfT[:, k, :],
                                                 in_=tph[:], func=AF.Copy)
                        for v in range(vsh // 512):
                            psl = ps_a.tile([P, 512], f32, name="psq")
                            for k in range(ND):
                                nc.tensor.matmul(
                                    out=psl[:], lhsT=hfT[:, k, :],
                                    rhs=wlm_sb[:, k, v * 512:(v + 1) * 512],
                                    start=(k == 0), stop=(k == ND - 1))
                            osb = xp.tile([P, 512], f32, name="ev512")
                            if v % 2 == 0:
                                nc.scalar.activation(out=osb[:], in_=psl[:],
                                                     func=AF.Copy)
                            else:
                                nc.vector.tensor_copy(out=osb[:], in_=psl[:])
                            nc.sync.dma_start(
                                out=out_d[(bt * NT + tt) * P:
                                          (bt * NT + tt + 1) * P,
                                          v * 512:(v + 1) * 512],
                                in_=osb[:])
    nc.compile()
    return nc


_NC_CACHE = {}


def _get_nc(key):
    if key not in _NC_CACHE:
        hsh, dsh, vsh, nb_lm, coll = key
        _NC_CACHE[key] = build_nc(hsh, dsh, vsh, nb_lm, coll)
    return _NC_CACHE[key]


def kernel(**inputs) -> np.ndarray:
    nc = _get_nc((8, 2048, VSH, 4, True))
    maps = host_prepare(inputs, hsh=8, dsh=2048, vsh=VSH)
    res = run_bass_kernel_spmd(nc, maps, core_ids=list(range(8)),
                               trace=False)
    logits = np.concatenate([res.results[c]["out"] for c in range(8)], axis=1)
    return np.ascontiguousarray(
        logits[:, : V + 1].reshape(B, T, V + 1).astype(np.float32))


# revision 9
# speedup vs baseline: 1.0473x; 1.0473x over previous
"""BlockDiffusionDecoder (mBART-style 2-layer decoder + BD3LM self-attn mask)
on 8 Trainium2 NeuronCores.

Sharding: cores (2b, 2b+1) own batch element b (B=4 -> 8 cores).  Within a
pair, tensor-parallel over heads (8 of 16) and d_ff (2048 of 4096), with a
pair AllReduce after the o-projections and fc2.  The LM head is sharded over
vocab 8 ways (padded 32768 = 8 x 4096) after an AllGather of final hidden
states across the {even} / {odd} core groups.

Layouts: activations live in SBUF as [128 tokens, tile, feature]; transposed
copies ([feature-tile, token]) are built with PE transposes.  All matmuls run
in bf16 (full PE rate); residual stream / LN / softmax stats stay fp32.
Weights are shipped from host pre-tiled and pre-cast to bf16.
"""
import sys

if "/opt/trn_rl_repo" not in sys.path:
    sys.path.insert(0, "/opt/trn_rl_repo")

import contextlib

import ml_dtypes
import numpy as np

import concourse.bass as bass
import concourse.bacc as bacc
import concourse.tile as tile
from concourse import mybir
from concourse.bass_utils import run_bass_kernel_spmd
from concourse.masks import make_identity

P = 128
B, D, H, NL, DFF, V, S = 4, 1024, 16, 2, 4096, 32000, 128
T = 1024
HD = D // H          # 64
BLK = 4
VP = 32768           # padded vocab (32001 -> 8*4096)
VSH = VP // 8        # vocab shard per core
NT = T // P          # 8 token tiles
ND = D // P          # 8 feature tiles
EMB_SCALE = 32.0     # sqrt(D)
FMIN = float(np.finfo(np.float32).min)
BF = ml_dtypes.bfloat16

f32 = mybir.dt.float32
bf16 = mybir.dt.bfloat16
i32 = mybir.dt.int32
AF = mybir.ActivationFunctionType
ALU = mybir.AluOpType
AX = mybir.AxisListType


def _rhs_tile(w_t: np.ndarray, nchunk: int) -> np.ndarray:
    """[d_in, d_out] -> [n_chunks, 128, k_tiles, nchunk] bf16, so the DMA of
    one n-chunk is contiguous per partition (k-major, n-minor)."""
    d_in, d_out = w_t.shape
    kt = d_in // P
    nc_ = d_out // nchunk
    return np.ascontiguousarray(
        w_t.reshape(kt, P, nc_, nchunk).transpose(2, 1, 0, 3).astype(BF))


def host_prepare(inputs: dict, hsh: int, dsh: int, vsh: int):
    """Build per-core input maps. hsh: heads/core, dsh: d_ff/core."""
    tp = 16 // hsh
    ids = np.asarray(inputs["input_ids"])
    enc = np.asarray(inputs["enc_hidden"], dtype=np.float32)
    emask = np.asarray(inputs["enc_mask"])
    emb = np.ascontiguousarray(np.asarray(inputs["embed_tokens"], np.float32))
    pos = np.ascontiguousarray(np.asarray(inputs["pos_embed"], np.float32))
    attn_w = np.asarray(inputs["attn_w"], np.float32)
    attn_b = np.asarray(inputs["attn_b"], np.float32)
    ln_w = np.asarray(inputs["ln_w"], np.float32)
    ln_b = np.asarray(inputs["ln_b"], np.float32)
    fc1_w = np.asarray(inputs["fc1_w"], np.float32)
    fc1_b = np.asarray(inputs["fc1_b"], np.float32)
    fc2_w = np.asarray(inputs["fc2_w"], np.float32)
    fc2_b = np.asarray(inputs["fc2_b"], np.float32)
    lm_w = np.asarray(inputs["lm_head_w"], np.float32)

    lm_pad = np.zeros((VP, D), np.float32)
    lm_pad[: V + 1] = lm_w
    lm_t = lm_pad.T  # [D, VP]

    n_cores = 8 if tp == 2 else 1
    maps = []
    for c in range(n_cores):
        b_ = c // tp
        j = c % tp
        hs = slice(j * hsh * HD, (j + 1) * hsh * HD)
        ds_ = slice(j * dsh, (j + 1) * dsh)
        vs_ = slice(c * vsh, (c + 1) * vsh) if tp == 2 else slice(0, vsh)
        m = {
            "ids": ids[b_].reshape(T, 1).astype(np.int32),
            "emb": emb,
            "pos": pos,
            "encT": np.ascontiguousarray(enc[b_].T.astype(BF)),   # [D, S]
            "cmask": ((1.0 - emask[b_].astype(np.float32)) * FMIN)
            .reshape(1, S),
            "lnemb": np.stack([np.asarray(inputs["ln_emb_s"], np.float32),
                               np.asarray(inputs["ln_emb_b"], np.float32)]),
            "lnfin": np.stack([np.asarray(inputs["final_ln_s"], np.float32),
                               np.asarray(inputs["final_ln_b"], np.float32)]),
            "lnw": ln_w, "lnb": ln_b,
            "wlm": np.ascontiguousarray(
                lm_t[:, vs_].reshape(ND, P, vsh).transpose(1, 0, 2)
                .astype(BF)),
        }
        for l in range(NL):
            for a, tag in ((0, "s"), (1, "c")):
                wq, wk, wv, wo = attn_w[l, a]
                bq, bk, bv, bo = attn_b[l, a]
                m[f"wq{tag}{l}"] = _rhs_tile(wq.T[:, hs], P)
                m[f"wk{tag}{l}"] = _rhs_tile(wk.T[:, hs], P)
                m[f"wv{tag}{l}"] = _rhs_tile(wv.T[:, hs], hsh * HD)
                m[f"wo{tag}{l}"] = _rhs_tile(wo.T[hs, :], D // 2)
                mh = hsh * HD // P
                m[f"bq{tag}{l}"] = np.ascontiguousarray(
                    bq[hs].reshape(mh, P).T)
                m[f"bk{tag}{l}"] = np.ascontiguousarray(
                    bk[hs].reshape(mh, P).T)
                m[f"bv{tag}{l}"] = bv[hs].reshape(1, hsh * HD).copy()
                m[f"bo{tag}{l}"] = (bo / tp).reshape(1, D).copy()
            m[f"wf1{l}"] = _rhs_tile(fc1_w[l].T[:, ds_], 512)
            m[f"bf1{l}"] = fc1_b[l][ds_].reshape(1, dsh).copy()
            m[f"wf2{l}"] = np.ascontiguousarray(
                fc2_w[l].T[ds_, :].reshape(dsh // 512, 4, P, D)
                .transpose(0, 2, 1, 3).astype(BF))  # [dffc, p, kk, D]
            m[f"bf2{l}"] = (fc2_b[l] / tp).reshape(1, D).copy()
        maps.append(m)
    return maps


def _mask_consts():
    i = np.arange(P)
    diag = np.where((i[:, None] // BLK) == (i[None, :] // BLK), 0.0, FMIN)
    tri_s = np.where((i[:, None] // BLK) > (i[None, :] // BLK), 0.0, FMIN)
    tri_i = np.where((i[:, None] // BLK) >= (i[None, :] // BLK), 0.0, FMIN)
    return (diag.astype(np.float32), tri_s.astype(np.float32),
            tri_i.astype(np.float32))


def build_nc(hsh=8, dsh=2048, vsh=VSH, nb_lm=4, collectives=True,
             gelu=AF.Gelu_apprx_tanh):
    tp = 16 // hsh
    MH = hsh * HD // P        # d_out tiles for q/k/v shard
    KO = MH                   # k-tiles for o-proj lhs
    NDC = dsh // 512          # dff chunks
    nc = bacc.Bacc(num_devices=8 if collectives else None, trn_type="TRN2")

    ids_d = nc.dram_tensor("ids", [T, 1], i32, kind="ExternalInput")
    emb_d = nc.dram_tensor("emb", [V + 1, D], f32, kind="ExternalInput")
    pos_d = nc.dram_tensor("pos", [T, D], f32, kind="ExternalInput")
    encT_d = nc.dram_tensor("encT", [D, S], bf16, kind="ExternalInput")
    cmask_d = nc.dram_tensor("cmask", [1, S], f32, kind="ExternalInput")
    lnemb_d = nc.dram_tensor("lnemb", [2, D], f32, kind="ExternalInput")
    lnfin_d = nc.dram_tensor("lnfin", [2, D], f32, kind="ExternalInput")
    lnw_d = nc.dram_tensor("lnw", [NL, 3, D], f32, kind="ExternalInput")
    lnb_d = nc.dram_tensor("lnb", [NL, 3, D], f32, kind="ExternalInput")
    wlm_d = nc.dram_tensor("wlm", [P, ND, vsh], bf16, kind="ExternalInput")
    wd, bd = {}, {}
    for l in range(NL):
        for tg in ("s", "c"):
            wd[f"wq{tg}{l}"] = nc.dram_tensor(
                f"wq{tg}{l}", [MH, P, ND, P], bf16, kind="ExternalInput")
            wd[f"wk{tg}{l}"] = nc.dram_tensor(
                f"wk{tg}{l}", [MH, P, ND, P], bf16, kind="ExternalInput")
            wd[f"wv{tg}{l}"] = nc.dram_tensor(
                f"wv{tg}{l}", [1, P, ND, hsh * HD], bf16,
                kind="ExternalInput")
            wd[f"wo{tg}{l}"] = nc.dram_tensor(
                f"wo{tg}{l}", [2, P, KO, D // 2], bf16, kind="ExternalInput")
            bd[f"bq{tg}{l}"] = nc.dram_tensor(
                f"bq{tg}{l}", [P, MH], f32, kind="ExternalInput")
            bd[f"bk{tg}{l}"] = nc.dram_tensor(
                f"bk{tg}{l}", [P, MH], f32, kind="ExternalInput")
            bd[f"bv{tg}{l}"] = nc.dram_tensor(
                f"bv{tg}{l}", [1, hsh * HD], f32, kind="ExternalInput")
            bd[f"bo{tg}{l}"] = nc.dram_tensor(
                f"bo{tg}{l}", [1, D], f32, kind="ExternalInput")
        wd[f"wf1{l}"] = nc.dram_tensor(
            f"wf1{l}", [NDC, P, ND, 512], bf16, kind="ExternalInput")
        bd[f"bf1{l}"] = nc.dram_tensor(
            f"bf1{l}", [1, dsh], f32, kind="ExternalInput")
        wd[f"wf2{l}"] = nc.dram_tensor(
            f"wf2{l}", [NDC, P, 4, D], bf16, kind="ExternalInput")
        bd[f"bf2{l}"] = nc.dram_tensor(
            f"bf2{l}", [1, D], f32, kind="ExternalInput")
    out_d = nc.dram_tensor("out", [nb_lm * T, vsh], f32,
                           kind="ExternalOutput")

    mdiag_np, mtris_np, mtrii_np = _mask_consts()
    mdiag_d = nc.inline_tensor(mdiag_np, "mdiag")
    mtris_d = nc.inline_tensor(mtris_np, "mtris")
    mtrii_d = nc.inline_tensor(mtrii_np, "mtrii")

    PAIRS = [[0, 1], [2, 3], [4, 5], [6, 7]]
    EVENODD = [[0, 2, 4, 6], [1, 3, 5, 7]]

    def bcast(ap_1d, p=P):
        return bass.AP(tensor=ap_1d.tensor, offset=ap_1d.offset,
                       ap=[[0, p]] + list(ap_1d.ap))

    with tile.TileContext(nc) as tc:
        gctx = contextlib.ExitStack()
        with gctx:
            consts = gctx.enter_context(tc.tile_pool(name="consts", bufs=1))
            small = gctx.enter_context(tc.tile_pool(name="small", bufs=4))
            sp = gctx.enter_context(tc.tile_pool(name="sp", bufs=2))
            xp = gctx.enter_context(tc.tile_pool(name="xp", bufs=3))
            xb = gctx.enter_context(tc.tile_pool(name="xb", bufs=2))
            dram = gctx.enter_context(
                tc.tile_pool(name="dram", bufs=1, space="DRAM"))
            ps_a = gctx.enter_context(
                tc.tile_pool(name="ps_a", bufs=4, space="PSUM"))
            ps_av = gctx.enter_context(
                tc.tile_pool(name="ps_av", bufs=2, space="PSUM"))
            ps_tr = gctx.enter_context(
                tc.tile_pool(name="ps_tr", bufs=2, space="PSUM"))

            ident = consts.tile([P, P], bf16)
            make_identity(nc, ident[:])
            eps_t = consts.tile([P, 1], f32)
            nc.vector.memset(eps_t[:], 1e-5)
            mdiag = consts.tile([P, P], f32)
            nc.sync.dma_start(out=mdiag[:], in_=mdiag_d[:])
            mtris = consts.tile([P, P], f32)
            nc.sync.dma_start(out=mtris[:], in_=mtris_d[:])
            mtrii = consts.tile([P, P], f32)
            nc.sync.dma_start(out=mtrii[:], in_=mtrii_d[:])
            cmask_b = consts.tile([P, S], f32)
            nc.sync.dma_start(out=cmask_b[:], in_=bcast(cmask_d[0]))

            cc_in = [dram.tile([T, D // 2], f32, name=f"cc_in{i}")
                     for i in range(2)]
            cc_out = ([dram.tile([T, D // 2], f32, name=f"cc_out{i}")
                       for i in range(2)] if collectives else cc_in)
            ccr_in = [dram.tile([T // 2, D], f32, name=f"ccr_in{i}")
                      for i in range(2)]
            ccr_out = ([dram.tile([T // 2, D], f32, name=f"ccr_out{i}")
                        for i in range(2)] if collectives else ccr_in)
            ccf_in = [dram.tile([T // 2, D], f32, name=f"ccf_in{i}")
                      for i in range(2)]
            ccf_out = ([dram.tile([nb_lm, T // 2, D], f32,
                                  name=f"ccf_out{i}") for i in range(2)]
                       if collectives else ccf_in)

            def ln_tile(src_ap, dst_ap, s_b, b_b):
                st = small.tile([P, 2, 6], f32, name="lnstats")
                nc.vector.bn_stats(out=st[:, 0, :], in_=src_ap[:, 0:512])
                nc.vector.bn_stats(out=st[:, 1, :], in_=src_ap[:, 512:1024])
                mv = small.tile([P, 2], f32, name="lnmv")
                nc.vector.bn_aggr(out=mv[:], in_=st[:])
                rstd = small.tile([P, 1], f32, name="lnrstd")
                nc.scalar.activation(out=rstd[:], in_=mv[:, 1:2],
                                     func=AF.Sqrt, bias=eps_t[:])
                nc.vector.reciprocal(out=rstd[:], in_=rstd[:])
                tmp = xp.tile([P, D], f32, name="xrow")
                nc.vector.tensor_scalar(out=tmp[:], in0=src_ap,
                                        scalar1=mv[:, 0:1], scalar2=rstd[:],
                                        op0=ALU.subtract, op1=ALU.mult)
                nc.vector.tensor_tensor(out=tmp[:], in0=tmp[:], in1=s_b,
                                        op=ALU.mult)
                nc.vector.tensor_tensor(out=dst_ap, in0=tmp[:], in1=b_b,
                                        op=ALU.add)

            # ---------------- stack phase ----------------
            sctx = contextlib.ExitStack()
            with sctx:
                pers = sctx.enter_context(tc.tile_pool(name="pers", bufs=1))
                wp = sctx.enter_context(tc.tile_pool(name="wp", bufs=2))
                lnp = sctx.enter_context(tc.tile_pool(name="lnp", bufs=1))

                h = pers.tile([P, NT, D], f32)
                encT = pers.tile([P, ND, S], bf16)
                nc.sync.dma_start(
                    out=encT[:],
                    in_=encT_d.rearrange("(k p) s -> p k s", p=P))

                def ln_consts(s_src, b_src):
                    s_b = lnp.tile([P, D], f32, name="ln_s")
                    nc.sync.dma_start(out=s_b[:], in_=bcast(s_src))
                    b_b = lnp.tile([P, D], f32, name="ln_b")
                    nc.sync.dma_start(out=b_b[:], in_=bcast(b_src))
                    return s_b, b_b

                def ln_to_xT(dst_xT, s_b, b_b):
                    """x = LN(h) (bf16) then xT[:, k, tt*128:] = T(x)."""
                    for tt in range(NT):
                        xt_ = xb.tile([P, D], bf16, name="xbrow")
                        ln_tile(h[:, tt, :], xt_[:], s_b[:], b_b[:])
                        for k in range(ND):
                            tp_ = ps_tr.tile([P, P], bf16, name="trps")
                            nc.tensor.transpose(
                                tp_[:], xt_[:, k * P:(k + 1) * P], ident[:])
                            if k % 2 == 0:
                                nc.scalar.activation(
                                    out=dst_xT[:, k, tt * P:(tt + 1) * P],
                                    in_=tp_[:], func=AF.Copy)
                            else:
                                nc.vector.tensor_copy(
                                    out=dst_xT[:, k, tt * P:(tt + 1) * P],
                                    in_=tp_[:])

                def proj_qk(dst, w_key, b_sb, scale, src_xT):
                    """dst[pd, m, t] = (xT.T @ W)^T with bias (+opt scale)."""
                    for m in range(MH):
                        wch = wp.tile([P, ND, P], bf16, name="wch")
                        nc.sync.dma_start(out=wch[:], in_=wd[w_key][m])
                        for half in range(2):
                            psq = ps_a.tile([P, 512], f32, name="psq")
                            for k in range(ND):
                                nc.tensor.matmul(
                                    out=psq[:],
                                    lhsT=wch[:, k, :],
                                    rhs=src_xT[:, k,
                                               half * 512:(half + 1) * 512],
                                    start=(k == 0), stop=(k == ND - 1))
                            if scale is None:
                                nc.vector.tensor_scalar(
                                    out=dst[:, m, half * 512:(half + 1) * 512],
                                    in0=psq[:], scalar1=b_sb[:, m:m + 1],
                                    scalar2=None, op0=ALU.add)
                            else:
                                nc.vector.tensor_scalar(
                                    out=dst[:, m, half * 512:(half + 1) * 512],
                                    in0=psq[:], scalar1=b_sb[:, m:m + 1],
                                    scalar2=scale, op0=ALU.add, op1=ALU.mult)

                def oproj_ar_update(src_oT, wo_key, bo_key):
                    """o-proj partial -> AllReduce (per column half,
                    pipelined) -> h update."""
                    bo_b = lnp.tile([P, D], f32, name="bo_b")
                    nc.sync.dma_start(out=bo_b[:], in_=bcast(bd[bo_key][0]))
                    for half in range(2):
                        wch = wp.tile([P, KO, 512], bf16, name="wch")
                        nc.sync.dma_start(out=wch[:], in_=wd[wo_key][half])
                        for tt in range(NT):
                            pso = ps_a.tile([P, 512], f32, name="psq")
                            for k in range(KO):
                                nc.tensor.matmul(
                                    out=pso[:],
                                    lhsT=src_oT[:, k, tt * P:(tt + 1) * P],
                                    rhs=wch[:, k, :],
                                    start=(k == 0), stop=(k == KO - 1))
                            ev = xp.tile([P, 512], f32, name="ev512")
                            if tt % 2 == 0:
                                nc.scalar.activation(out=ev[:], in_=pso[:],
                                                     func=AF.Copy)
                            else:
                                nc.vector.tensor_copy(out=ev[:], in_=pso[:])
                            nc.sync.dma_start(
                                out=cc_in[half][tt * P:(tt + 1) * P, :],
                                in_=ev[:])
                        if collectives:
                            nc.gpsimd.collective_compute(
                                "AllReduce", ALU.add, replica_groups=PAIRS,
                                ins=[cc_in[half][:]], outs=[cc_out[half][:]])
                    for tt in range(NT):
                        for half in range(2):
                            dtile = xp.tile([P, 512], f32, name="ev512")
                            nc.sync.dma_start(
                                out=dtile[:],
                                in_=cc_out[half][tt * P:(tt + 1) * P, :])
                            nc.vector.tensor_tensor(
                                out=dtile[:], in0=dtile[:],
                                in1=bo_b[:, half * 512:(half + 1) * 512],
                                op=ALU.add)
                            nc.vector.tensor_tensor(
                                out=h[:, tt, half * 512:(half + 1) * 512],
                                in0=h[:, tt, half * 512:(half + 1) * 512],
                                in1=dtile[:], op=ALU.add)

                # ---- embed + emb LN ----
                lnes, lneb = ln_consts(lnemb_d[0], lnemb_d[1])
                for tt in range(NT):
                    idt = small.tile([P, 1], i32, name="idt")
                    nc.sync.dma_start(out=idt[:],
                                      in_=ids_d[tt * P:(tt + 1) * P])
                    g = xp.tile([P, D], f32, name="xrow")
                    nc.gpsimd.indirect_dma_start(
                        out=g[:], out_offset=None, in_=emb_d[:],
                        in_offset=bass.IndirectOffsetOnAxis(
                            ap=idt[:, :1], axis=0))
                    pt = xp.tile([P, D], f32, name="xrow")
                    nc.sync.dma_start(out=pt[:],
                                      in_=pos_d[tt * P:(tt + 1) * P])
                    nc.vector.tensor_scalar(out=g[:], in0=g[:],
                                            scalar1=EMB_SCALE, scalar2=None,
                                            op0=ALU.mult)
                    nc.vector.tensor_tensor(out=h[:, tt, :], in0=g[:],
                                            in1=pt[:], op=ALU.add)
                    ln_tile(h[:, tt, :], h[:, tt, :], lnes[:], lneb[:])

                for l in range(NL):
                    # ======== self attention ========
                    lns, lnbb = ln_consts(lnw_d[l, 0], lnb_d[l, 0])
                    xT = pers.tile([P, ND, T], bf16, name="xT", tag="xT")
                    ln_to_xT(xT, lns, lnbb)

                    qT = pers.tile([P, MH, T], bf16, name="qT", tag="qT")
                    kT = pers.tile([P, MH, T], bf16, name="kT", tag="kT")
                    vv = pers.tile([P, NT, hsh * HD], bf16, name="vv",
                                   tag="vv")
                    bq_sb = small.tile([P, MH], f32, name="bq")
                    nc.sync.dma_start(out=bq_sb[:], in_=bd[f"bqs{l}"][:])
                    bk_sb = small.tile([P, MH], f32, name="bk")
                    nc.sync.dma_start(out=bk_sb[:], in_=bd[f"bks{l}"][:])
                    bv_b = lnp.tile([P, hsh * HD], f32, name="bv_b")
                    nc.sync.dma_start(out=bv_b[:],
                                      in_=bcast(bd[f"bvs{l}"][0]))

                    proj_qk(qT, f"wqs{l}", bq_sb, 0.125, xT)
                    proj_qk(kT, f"wks{l}", bk_sb, None, xT)
                    wch_v = wp.tile([P, ND, hsh * HD], bf16, name="wch")
                    nc.sync.dma_start(out=wch_v[:], in_=wd[f"wvs{l}"][0])
                    for tt in range(NT):
                        psv = ps_a.tile([P, 512], f32, name="psq")
                        for k in range(ND):
                            nc.tensor.matmul(
                                out=psv[:],
                                lhsT=xT[:, k, tt * P:(tt + 1) * P],
                                rhs=wch_v[:, k, :],
                                start=(k == 0), stop=(k == ND - 1))
                        nc.vector.tensor_tensor(out=vv[:, tt, :], in0=psv[:],
                                                in1=bv_b[:], op=ALU.add)

                    # scores + softmax + AV per head
                    oT = pers.tile([P, KO, T], bf16, name="oT", tag="oT")
                    for hl in range(hsh):
                        prow = slice((hl % 2) * 64, (hl % 2) * 64 + 64)
                        mq = hl // 2
                        ptile = sp.tile([P, 5, T], bf16, name="ptile")
                        for g in range(NT):
                            width = (g + 2) * P if g < 4 else (g - 3) * P
                            ssb = sp.tile([P, 640], f32, name="srow")
                            qst = qT[prow, mq, g * P:(g + 1) * P]
                            if g < 4:
                                psd = ps_a.tile([P, 512], f32, name="psq")
                                nc.tensor.matmul(
                                    out=psd[:, 0:P], lhsT=qst,
                                    rhs=kT[prow, mq, g * P:(g + 1) * P],
                                    start=True, stop=True)
                                pss = ps_a.tile([P, 512], f32, name="psq")
                                nc.tensor.matmul(
                                    out=pss[:, 0:width - P], lhsT=qst,
                                    rhs=kT[prow, mq, 512:512 + width - P],
                                    start=True, stop=True)
                                nc.vector.tensor_tensor(
                                    out=ssb[:, 0:P], in0=psd[:, 0:P],
                                    in1=mdiag[:], op=ALU.add)
                                if g > 0:
                                    nc.scalar.activation(
                                        out=ssb[:, P:width - P],
                                        in_=pss[:, 0:width - 2 * P],
                                        func=AF.Copy)
                                nc.vector.tensor_tensor(
                                    out=ssb[:, width - P:width],
                                    in0=pss[:, width - 2 * P:width - P],
                                    in1=mtris[:], op=ALU.add)
                            else:
                                pss = ps_a.tile([P, 512], f32, name="psq")
                                nc.tensor.matmul(
                                    out=pss[:, 0:width], lhsT=qst,
                                    rhs=kT[prow, mq, 512:512 + width],
                                    start=True, stop=True)
                                if width > P:
                                    nc.scalar.activation(
                                        out=ssb[:, 0:width - P],
                                        in_=pss[:, 0:width - P], func=AF.Copy)
                                nc.vector.tensor_tensor(
                                    out=ssb[:, width - P:width],
                                    in0=pss[:, width - P:width],
                                    in1=mtrii[:], op=ALU.add)
                            negmax = small.tile([P, 1], f32, name="negmax")
                            nc.vector.tensor_reduce(
                                out=negmax[:], in_=ssb[:, 0:width],
                                axis=AX.X, op=ALU.max, negate=True)
                            probs = sp.tile([P, 640], bf16, name="brow")
                            sums = small.tile([P, 1], f32, name="sums")
                            nc.scalar.activation(
                                out=probs[:, 0:width], in_=ssb[:, 0:width],
                                func=AF.Exp, bias=negmax[:],
                                accum_out=sums[:])
                            recip = small.tile([P, 1], f32, name="recip")
                            nc.vector.reciprocal(out=recip[:], in_=sums[:])
                            nc.vector.tensor_scalar(
                                out=probs[:, 0:width], in0=probs[:, 0:width],
                                scalar1=recip[:], scalar2=None, op0=ALU.mult)
                            if g < 4:
                                chunks = [(4, 0)] + [(mm, (mm + 1) * P)
                                                     for mm in range(g + 1)]
                            else:
                                chunks = [(mm, mm * P)
                                          for mm in range(g - 3)]
                            for ci, (slot, coff) in enumerate(chunks):
                                tpp = ps_tr.tile([P, P], bf16, name="trps")
                                nc.tensor.transpose(
                                    tpp[:], probs[:, coff:coff + P],
                                    ident[:])
                                if (g + ci) % 2 == 0:
                                    nc.scalar.activation(
                                        out=ptile[:, slot,
                                                  g * P:(g + 1) * P],
                                        in_=tpp[:], func=AF.Copy)
                                else:
                                    nc.vector.tensor_copy(
                                        out=ptile[:, slot,
                                                  g * P:(g + 1) * P],
                                        in_=tpp[:])
                        pav_lo = ps_av.tile([64, 512], f32, name="pav")
                        pav_hi = ps_av.tile([64, 512], f32, name="pav")
                        for mm in range(4):
                            nc.tensor.matmul(
                                out=pav_lo[:, mm * P:512],
                                lhsT=vv[:, 4 + mm, hl * HD:(hl + 1) * HD],
                                rhs=ptile[:, mm, mm * P:512],
                                start=(mm == 0), stop=False)
                            nc.tensor.matmul(
                                out=pav_hi[:, mm * P:512],
                                lhsT=vv[:, 4 + mm, hl * HD:(hl + 1) * HD],
                                rhs=ptile[:, mm, 512 + mm * P:T],
                                start=(mm == 0), stop=(mm == 3))
                        for g in range(4):
                            nc.tensor.matmul(
                                out=pav_lo[:, g * P:(g + 1) * P],
                                lhsT=vv[:, g, hl * HD:(hl + 1) * HD],
                                rhs=ptile[:, 4, g * P:(g + 1) * P],
                                start=False, stop=(g == 3))
                        if hl % 2 == 0:
                            nc.scalar.activation(out=oT[prow, mq, 0:512],
                                                 in_=pav_lo[:], func=AF.Copy)
                            nc.vector.tensor_copy(out=oT[prow, mq, 512:T],
                                                  in_=pav_hi[:])
                        else:
                            nc.vector.tensor_copy(out=oT[prow, mq, 0:512],
                                                  in_=pav_lo[:])
                            nc.scalar.activation(out=oT[prow, mq, 512:T],
                                                 in_=pav_hi[:], func=AF.Copy)
                    oproj_ar_update(oT, f"wos{l}", f"bos{l}")

                    # ======== cross attention ========
                    lns2, lnb2 = ln_consts(lnw_d[l, 1], lnb_d[l, 1])
                    x2T = pers.tile([P, ND, T], bf16, name="x2T", tag="xT")
                    ln_to_xT(x2T, lns2, lnb2)
                    q2T = pers.tile([P, MH, T], bf16, name="q2T", tag="qT")
                    bq2_sb = small.tile([P, MH], f32, name="bq")
                    nc.sync.dma_start(out=bq2_sb[:], in_=bd[f"bqc{l}"][:])
                    bk2_sb = small.tile([P, MH], f32, name="bk")
                    nc.sync.dma_start(out=bk2_sb[:], in_=bd[f"bkc{l}"][:])
                    bv2_b = lnp.tile([P, hsh * HD], f32, name="bv_b")
                    nc.sync.dma_start(out=bv2_b[:],
                                      in_=bcast(bd[f"bvc{l}"][0]))
                    proj_qk(q2T, f"wqc{l}", bq2_sb, 0.125, x2T)

                    wch_k = wp.tile([P, ND, MH, P], bf16, name="wch")
                    for m in range(MH):
                        nc.sync.dma_start(out=wch_k[:, :, m, :],
                                          in_=wd[f"wkc{l}"][m])
                    kenc_raw = sp.tile([S, hsh * HD], bf16, name="kenc", bufs=1)
                    psk = ps_a.tile([P, 512], f32, name="psq")
                    for k in range(ND):
                        nc.tensor.matmul(out=psk[:], lhsT=encT[:, k, :],
                                         rhs=wch_k[:, k, :, :],
                                         start=(k == 0), stop=(k == ND - 1))
                    nc.scalar.activation(out=kenc_raw[:], in_=psk[:],
                                         func=AF.Copy)
                    kTe = sp.tile([P, MH, S], bf16, name="kTe", bufs=1)
                    for m in range(MH):
                        tpk = ps_tr.tile([P, P], bf16, name="trps")
                        nc.tensor.transpose(
                            tpk[:], kenc_raw[:, m * P:(m + 1) * P], ident[:])
                        nc.vector.tensor_scalar(
                            out=kTe[:, m, :], in0=tpk[:],
                            scalar1=bk2_sb[:, m:m + 1], scalar2=None,
                            op0=ALU.add)
                    wch_v2 = wp.tile([P, ND, hsh * HD], bf16, name="wch")
                    nc.sync.dma_start(out=wch_v2[:], in_=wd[f"wvc{l}"][0])
                    venc = sp.tile([S, hsh * HD], bf16, name="venc", bufs=1)
                    psv2 = ps_a.tile([P, 512], f32, name="psq")
                    for k in range(ND):
                        nc.tensor.matmul(out=psv2[:], lhsT=encT[:, k, :],
                                         rhs=wch_v2[:, k, :],
                                         start=(k == 0), stop=(k == ND - 1))
                    nc.vector.tensor_tensor(out=venc[:], in0=psv2[:],
                                            in1=bv2_b[:], op=ALU.add)

                    o2T = pers.tile([P, KO, T], bf16, name="o2T", tag="oT")
                    for hl in range(hsh):
                        prow = slice((hl % 2) * 64, (hl % 2) * 64 + 64)
                        mq = hl // 2
                        p2tile = sp.tile([S, T], bf16, name="ptile")
                        for tt in range(NT):
                            ps2 = ps_a.tile([P, 512], f32, name="psq")
                            nc.tensor.matmul(
                                out=ps2[:, 0:S],
                                lhsT=q2T[prow, mq, tt * P:(tt + 1) * P],
                                rhs=kTe[prow, mq, :], start=True, stop=True)
                            s2 = sp.tile([P, S], f32, name="srow")
                            nc.vector.tensor_tensor(out=s2[:],
                                                    in0=ps2[:, 0:S],
                                                    in1=cmask_b[:],
                                                    op=ALU.add)
                            negmax = small.tile([P, 1], f32, name="negmax")
                            nc.vector.tensor_reduce(
                                out=negmax[:], in_=s2[:], axis=AX.X,
                                op=ALU.max, negate=True)
                            probs2 = sp.tile([P, S], bf16, name="brow")
                            sums = small.tile([P, 1], f32, name="sums")
                            nc.scalar.activation(
                                out=probs2[:], in_=s2[:], func=AF.Exp,
                                bias=negmax[:], accum_out=sums[:])
                            recip = small.tile([P, 1], f32, name="recip")
                            nc.vector.reciprocal(out=recip[:], in_=sums[:])
                            nc.vector.tensor_scalar(
                                out=probs2[:], in0=probs2[:],
                                scalar1=recip[:], scalar2=None, op0=ALU.mult)
                            tpp = ps_tr.tile([P, P], bf16, name="trps")
                            nc.tensor.transpose(tpp[:], probs2[:], ident[:])
                            nc.scalar.activation(
                                out=p2tile[:, tt * P:(tt + 1) * P],
                                in_=tpp[:], func=AF.Copy)
                        for half in range(2):
                            pav2 = ps_av.tile([64, 512], f32, name="pav")
                            nc.tensor.matmul(
                                out=pav2[:],
                                lhsT=venc[:, hl * HD:(hl + 1) * HD],
                                rhs=p2tile[:, half * 512:(half + 1) * 512],
                                start=True, stop=True)
                            nc.scalar.activation(
                                out=o2T[prow, mq, half * 512:(half + 1) * 512],
                                in_=pav2[:], func=AF.Copy)
                    oproj_ar_update(o2T, f"woc{l}", f"boc{l}")

                    # ======== FFN ========
                    lns3, lnb3 = ln_consts(lnw_d[l, 2], lnb_d[l, 2])
                    x3T = pers.tile([P, ND, T], bf16, name="x3T", tag="xT")
                    ln_to_xT(x3T, lns3, lnb3)
                    bf1_b = lnp.tile([P, dsh], f32, name="bf1_b")
                    nc.sync.dma_start(out=bf1_b[:],
                                      in_=bcast(bd[f"bf1{l}"][0]))
                    bf2_b = lnp.tile([P, D], f32, name="bo_b")
                    nc.sync.dma_start(out=bf2_b[:],
                                      in_=bcast(bd[f"bf2{l}"][0]))
                    acc = pers.tile([P, NT, D], f32, name="acc", tag="oT")
                    for dffc in range(NDC):
                        f1c = wp.tile([P, ND, 512], bf16, name="wch")
                        nc.sync.dma_start(out=f1c[:],
                                          in_=wd[f"wf1{l}"][dffc])
                        f2c = wp.tile([P, 4, D], bf16, name="wch")
                        nc.sync.dma_start(out=f2c[:],
                                          in_=wd[f"wf2{l}"][dffc])
                        for tt in range(NT):
                            psf = ps_a.tile([P, 512], f32, name="psq")
                            for k in range(ND):
                                nc.tensor.matmul(
                                    out=psf[:],
                                    lhsT=x3T[:, k, tt * P:(tt + 1) * P],
                                    rhs=f1c[:, k, :],
                                    start=(k == 0), stop=(k == ND - 1))
                            gact = xb.tile([P, 512], bf16, name="gact")
                            nc.vector.tensor_tensor(
                                out=gact[:], in0=psf[:],
                                in1=bf1_b[:, dffc * 512:(dffc + 1) * 512],
                                op=ALU.add)
                            nc.scalar.activation(out=gact[:], in_=gact[:],
                                                 func=gelu)
                            gT = xb.tile([P, 4, P], bf16, name="gT")
                            for kk in range(4):
                                tpg = ps_tr.tile([P, P], bf16, name="trps")
                                nc.tensor.transpose(
                                    tpg[:], gact[:, kk * P:(kk + 1) * P],
                                    ident[:])
                                nc.scalar.activation(out=gT[:, kk, :],
                                                     in_=tpg[:],
                                                     func=AF.Copy)
                            for n2 in range(2):
                                psf2 = ps_a.tile([P, 512], f32, name="psq")
                                for kk in range(4):
                                    nc.tensor.matmul(
                                        out=psf2[:], lhsT=gT[:, kk, :],
                                        rhs=f2c[:, kk,
                                                n2 * 512:(n2 + 1) * 512],
                                        start=(kk == 0), stop=(kk == 3))
                                if dffc == 0:
                                    nc.vector.tensor_copy(
                                        out=acc[:, tt,
                                                n2 * 512:(n2 + 1) * 512],
                                        in_=psf2[:])
                                else:
                                    nc.vector.tensor_tensor(
                                        out=acc[:, tt,
                                                n2 * 512:(n2 + 1) * 512],
                                        in0=acc[:, tt,
                                                n2 * 512:(n2 + 1) * 512],
                                        in1=psf2[:], op=ALU.add)
                    for rh in range(2):
                        for tt in range(4 * rh, 4 * rh + 4):
                            nc.sync.dma_start(
                                out=ccr_in[rh][(tt - 4 * rh) * P:
                                               (tt - 4 * rh + 1) * P, :],
                                in_=acc[:, tt, :])
                        if collectives:
                            nc.gpsimd.collective_compute(
                                "AllReduce", ALU.add, replica_groups=PAIRS,
                                ins=[ccr_in[rh][:]], outs=[ccr_out[rh][:]])
                    for tt in range(NT):
                        rh, tl = tt // 4, tt % 4
                        dtile = xp.tile([P, D], f32, name="xrow")
                        nc.sync.dma_start(
                            out=dtile[:],
                            in_=ccr_out[rh][tl * P:(tl + 1) * P, :])
                        nc.vector.tensor_tensor(out=dtile[:], in0=dtile[:],
                                                in1=bf2_b[:], op=ALU.add)
                        nc.vector.tensor_tensor(out=h[:, tt, :],
                                                in0=h[:, tt, :],
                                                in1=dtile[:], op=ALU.add)

                # ---- final LN -> ccf ----
                lnfs, lnfb = ln_consts(lnfin_d[0], lnfin_d[1])
                for rh in range(2):
                    for tl in range(4):
                        hf = xp.tile([P, D], f32, name="xrow")
                        ln_tile(h[:, 4 * rh + tl, :], hf[:], lnfs[:],
                                lnfb[:])
                        nc.sync.dma_start(
                            out=ccf_in[rh][tl * P:(tl + 1) * P, :],
                            in_=hf[:])
                    if collectives:
                        nc.gpsimd.collective_compute(
                            "AllGather", ALU.bypass, replica_groups=EVENODD,
                            ins=[ccf_in[rh][:]], outs=[ccf_out[rh][:]])
            # stack pools closed here

            # ---------------- LM head ----------------
            lctx = contextlib.ExitStack()
            with lctx:
                lmp = lctx.enter_context(tc.tile_pool(name="lmp", bufs=1))
                lmt = lctx.enter_context(tc.tile_pool(name="lmt", bufs=2))
                wlm_sb = lmp.tile([P, ND, vsh], bf16)
                for k in range(ND):
                    nc.sync.dma_start(out=wlm_sb[:, k, :], in_=wlm_d[:, k, :])
                for bt in range(nb_lm):
                    for tt in range(NT):
                        hft = xb.tile([P, D], bf16, name="xbrow")
                        rh, tl = tt // 4, tt % 4
                        src = (ccf_out[rh][bt, tl * P:(tl + 1) * P, :]
                               if collectives
                               else ccf_in[rh][tl * P:(tl + 1) * P, :])
                        nc.gpsimd.dma_start(out=hft[:], in_=src)
                        hfT = lmt.tile([P, ND, P], bf16, name="hfT")
                        for k in range(ND):
                            tph = ps_tr.tile([P, P], bf16, name="trps")
                            nc.tensor.transpose(
                                tph[:], hft[:, k * P:(k + 1) * P], ident[:])
                            if k % 2 == 0:
                                nc.scalar.activation(out=hfT[:, k, :],
                                                     in_=tph[:],
                                                     func=AF.Copy)
                            else:
                                nc.vector.tensor_copy(out=hfT[:, k, :],
                                                      in_=tph[:])
                        for v in range(vsh // 512):
                            psl = ps_a.tile([P, 512], f32, name="psq")
                            for k in range(ND):
                                nc.tensor.matmul(
                                    out=psl[:], lhsT=hfT[:, k, :],
                                    rhs=wlm_sb[:, k, v * 512:(v + 1) * 512],
                                    start=(k == 0), stop=(k == ND - 1))
                            osb = xp.tile([P, 512], f32, name="ev512")
                            if v % 2 == 0:
                                nc.scalar.activation(out=osb[:], in_=psl[:],
                                                     func=AF.Copy)
                            else:
                                nc.vector.tensor_copy(out=osb[:], in_=psl[:])
                            nc.sync.dma_start(
                                out=out_d[(bt * NT + tt) * P:
                                          (bt * NT + tt + 1) * P,
                                          v * 512:(v + 1) * 512],
                                in_=osb[:])
    nc.compile()
    return nc


_NC_CACHE = {}


def _get_nc(key):
    if key not in _NC_CACHE:
        hsh, dsh, vsh, nb_lm, coll = key
        _NC_CACHE[key] = build_nc(hsh, dsh, vsh, nb_lm, coll)
    return _NC_CACHE[key]


def kernel(**inputs) -> np.ndarray:
    nc = _get_nc((8, 2048, VSH, 4, True))
    maps = host_prepare(inputs, hsh=8, dsh=2048, vsh=VSH)
    res = run_bass_kernel_spmd(nc, maps, core_ids=list(range(8)),
                               trace=False)
    logits = np.concatenate([res.results[c]["out"] for c in range(8)], axis=1)
    return np.ascontiguousarray(
        logits[:, : V + 1].reshape(B, T, V + 1).astype(np.float32))


# revision 13
# speedup vs baseline: 1.2779x; 1.2202x over previous
"""BlockDiffusionDecoder (mBART-style 2-layer decoder + BD3LM self-attn mask)
on 8 Trainium2 NeuronCores.

Sharding: cores (2b, 2b+1) own batch element b (B=4 -> 8 cores).  Within a
pair, tensor-parallel over heads (8 of 16) and d_ff (2048 of 4096), with a
pair AllReduce after the o-projections and fc2.  The LM head is sharded over
vocab 8 ways (padded 32768 = 8 x 4096) after an AllGather of final hidden
states across the {even} / {odd} core groups.

Layouts: activations live in SBUF as [128 tokens, tile, feature]; transposed
copies ([feature-tile, token]) are built with PE transposes.  All matmuls run
in bf16 (full PE rate); residual stream / LN / softmax stats stay fp32.
Weights are shipped from host pre-tiled and pre-cast to bf16.
"""
import sys

if "/opt/trn_rl_repo" not in sys.path:
    sys.path.insert(0, "/opt/trn_rl_repo")

import contextlib

import ml_dtypes
import numpy as np

import concourse.bass as bass
import concourse.bacc as bacc
import concourse.tile as tile
from concourse import mybir
from concourse.bass_utils import run_bass_kernel_spmd
from concourse.masks import make_identity

P = 128
B, D, H, NL, DFF, V, S = 4, 1024, 16, 2, 4096, 32000, 128
T = 1024
HD = D // H          # 64
BLK = 4
VP = 32768           # padded vocab (32001 -> 8*4096)
VSH = VP // 8        # vocab shard per core
NT = T // P          # 8 token tiles
ND = D // P          # 8 feature tiles
EMB_SCALE = 32.0     # sqrt(D)
FMIN = float(np.finfo(np.float32).min)
BF = ml_dtypes.bfloat16

f32 = mybir.dt.float32
bf16 = mybir.dt.bfloat16
i32 = mybir.dt.int32
AF = mybir.ActivationFunctionType
ALU = mybir.AluOpType
AX = mybir.AxisListType


def _rhs_tile(w_t: np.ndarray, nchunk: int) -> np.ndarray:
    """[d_in, d_out] -> [n_chunks, 128, k_tiles, nchunk] bf16, so the DMA of
    one n-chunk is contiguous per partition (k-major, n-minor)."""
    d_in, d_out = w_t.shape
    kt = d_in // P
    nc_ = d_out // nchunk
    return np.ascontiguousarray(
        w_t.reshape(kt, P, nc_, nchunk).transpose(2, 1, 0, 3).astype(BF))


def host_prepare(inputs: dict, hsh: int, dsh: int, vsh: int):
    """Build per-core input maps. hsh: heads/core, dsh: d_ff/core."""
    tp = 16 // hsh
    ids = np.asarray(inputs["input_ids"])
    enc = np.asarray(inputs["enc_hidden"], dtype=np.float32)
    emask = np.asarray(inputs["enc_mask"])
    emb = np.ascontiguousarray(np.asarray(inputs["embed_tokens"], np.float32))
    pos = np.ascontiguousarray(np.asarray(inputs["pos_embed"], np.float32))
    attn_w = np.asarray(inputs["attn_w"], np.float32)
    attn_b = np.asarray(inputs["attn_b"], np.float32)
    ln_w = np.asarray(inputs["ln_w"], np.float32)
    ln_b = np.asarray(inputs["ln_b"], np.float32)
    fc1_w = np.asarray(inputs["fc1_w"], np.float32)
    fc1_b = np.asarray(inputs["fc1_b"], np.float32)
    fc2_w = np.asarray(inputs["fc2_w"], np.float32)
    fc2_b = np.asarray(inputs["fc2_b"], np.float32)
    lm_w = np.asarray(inputs["lm_head_w"], np.float32)

    lm_pad = np.zeros((VP, D), np.float32)
    lm_pad[: V + 1] = lm_w
    lm_t = lm_pad.T  # [D, VP]

    n_cores = 8 if tp == 2 else 1
    maps = []
    for c in range(n_cores):
        b_ = c // tp
        j = c % tp
        hs = slice(j * hsh * HD, (j + 1) * hsh * HD)
        ds_ = slice(j * dsh, (j + 1) * dsh)
        vs_ = slice(c * vsh, (c + 1) * vsh) if tp == 2 else slice(0, vsh)
        m = {
            "ids": ids[b_].reshape(T, 1).astype(np.int32),
            "emb": emb,
            "pos": pos,
            "encT": np.ascontiguousarray(enc[b_].T.astype(BF)),   # [D, S]
            "cmask": ((1.0 - emask[b_].astype(np.float32)) * FMIN)
            .reshape(1, S),
            "lnemb": np.stack([np.asarray(inputs["ln_emb_s"], np.float32),
                               np.asarray(inputs["ln_emb_b"], np.float32)]),
            "lnfin": np.stack([np.asarray(inputs["final_ln_s"], np.float32),
                               np.asarray(inputs["final_ln_b"], np.float32)]),
            "lnw": ln_w, "lnb": ln_b,
            "wlm": np.ascontiguousarray(
                lm_t[:, vs_].reshape(ND, P, vsh).transpose(1, 0, 2)
                .astype(BF)),
        }
        for l in range(NL):
            for a, tag in ((0, "s"), (1, "c")):
                wq, wk, wv, wo = attn_w[l, a]
                bq, bk, bv, bo = attn_b[l, a]
                m[f"wq{tag}{l}"] = _rhs_tile(wq.T[:, hs], P)
                m[f"wk{tag}{l}"] = _rhs_tile(wk.T[:, hs], P)
                m[f"wv{tag}{l}"] = _rhs_tile(wv.T[:, hs], hsh * HD)
                m[f"wo{tag}{l}"] = _rhs_tile(wo.T[hs, :], D // 2)
                mh = hsh * HD // P
                m[f"bq{tag}{l}"] = np.ascontiguousarray(
                    bq[hs].reshape(mh, P).T)
                m[f"bk{tag}{l}"] = np.ascontiguousarray(
                    bk[hs].reshape(mh, P).T)
                m[f"bv{tag}{l}"] = bv[hs].reshape(1, hsh * HD).copy()
                m[f"bo{tag}{l}"] = (bo / tp).reshape(1, D).copy()
            m[f"wf1{l}"] = _rhs_tile(fc1_w[l].T[:, ds_], P)
            m[f"bf1{l}"] = np.ascontiguousarray(
                fc1_b[l][ds_].reshape(dsh // P, P).T)
            m[f"wf2{l}"] = np.ascontiguousarray(
                fc2_w[l].T[ds_, :].reshape(dsh // P, P, 2, 512)
                .transpose(2, 1, 0, 3).astype(BF))  # [n2, p, kk, 512]
            m[f"bf2{l}"] = (fc2_b[l] / tp).reshape(1, D).copy()
        maps.append(m)
    return maps


def _mask_consts():
    i = np.arange(P)
    diag = np.where((i[:, None] // BLK) == (i[None, :] // BLK), 0.0, FMIN)
    tri_s = np.where((i[:, None] // BLK) > (i[None, :] // BLK), 0.0, FMIN)
    tri_i = np.where((i[:, None] // BLK) >= (i[None, :] // BLK), 0.0, FMIN)
    return (diag.astype(np.float32), tri_s.astype(np.float32),
            tri_i.astype(np.float32))


def build_nc(hsh=8, dsh=2048, vsh=VSH, nb_lm=4, collectives=True,
             gelu=AF.Gelu_apprx_tanh):
    tp = 16 // hsh
    MH = hsh * HD // P        # d_out tiles for q/k/v shard
    KO = MH                   # k-tiles for o-proj lhs
    NDC = dsh // 512          # dff chunks
    nc = bacc.Bacc(num_devices=8 if collectives else None, trn_type="TRN2")

    ids_d = nc.dram_tensor("ids", [T, 1], i32, kind="ExternalInput")
    emb_d = nc.dram_tensor("emb", [V + 1, D], f32, kind="ExternalInput")
    pos_d = nc.dram_tensor("pos", [T, D], f32, kind="ExternalInput")
    encT_d = nc.dram_tensor("encT", [D, S], bf16, kind="ExternalInput")
    cmask_d = nc.dram_tensor("cmask", [1, S], f32, kind="ExternalInput")
    lnemb_d = nc.dram_tensor("lnemb", [2, D], f32, kind="ExternalInput")
    lnfin_d = nc.dram_tensor("lnfin", [2, D], f32, kind="ExternalInput")
    lnw_d = nc.dram_tensor("lnw", [NL, 3, D], f32, kind="ExternalInput")
    lnb_d = nc.dram_tensor("lnb", [NL, 3, D], f32, kind="ExternalInput")
    wlm_d = nc.dram_tensor("wlm", [P, ND, vsh], bf16, kind="ExternalInput")
    wd, bd = {}, {}
    for l in range(NL):
        for tg in ("s", "c"):
            wd[f"wq{tg}{l}"] = nc.dram_tensor(
                f"wq{tg}{l}", [MH, P, ND, P], bf16, kind="ExternalInput")
            wd[f"wk{tg}{l}"] = nc.dram_tensor(
                f"wk{tg}{l}", [MH, P, ND, P], bf16, kind="ExternalInput")
            wd[f"wv{tg}{l}"] = nc.dram_tensor(
                f"wv{tg}{l}", [1, P, ND, hsh * HD], bf16,
                kind="ExternalInput")
            wd[f"wo{tg}{l}"] = nc.dram_tensor(
                f"wo{tg}{l}", [2, P, KO, D // 2], bf16, kind="ExternalInput")
            bd[f"bq{tg}{l}"] = nc.dram_tensor(
                f"bq{tg}{l}", [P, MH], f32, kind="ExternalInput")
            bd[f"bk{tg}{l}"] = nc.dram_tensor(
                f"bk{tg}{l}", [P, MH], f32, kind="ExternalInput")
            bd[f"bv{tg}{l}"] = nc.dram_tensor(
                f"bv{tg}{l}", [1, hsh * HD], f32, kind="ExternalInput")
            bd[f"bo{tg}{l}"] = nc.dram_tensor(
                f"bo{tg}{l}", [1, D], f32, kind="ExternalInput")
        wd[f"wf1{l}"] = nc.dram_tensor(
            f"wf1{l}", [dsh // P, P, ND, P], bf16, kind="ExternalInput")
        bd[f"bf1{l}"] = nc.dram_tensor(
            f"bf1{l}", [P, dsh // P], f32, kind="ExternalInput")
        wd[f"wf2{l}"] = nc.dram_tensor(
            f"wf2{l}", [2, P, dsh // P, 512], bf16, kind="ExternalInput")
        bd[f"bf2{l}"] = nc.dram_tensor(
            f"bf2{l}", [1, D], f32, kind="ExternalInput")
    out_d = nc.dram_tensor("out", [nb_lm * T, vsh], f32,
                           kind="ExternalOutput")

    mdiag_np, mtris_np, mtrii_np = _mask_consts()
    mdiag_d = nc.inline_tensor(mdiag_np, "mdiag")
    mtris_d = nc.inline_tensor(mtris_np, "mtris")
    mtrii_d = nc.inline_tensor(mtrii_np, "mtrii")

    PAIRS = [[0, 1], [2, 3], [4, 5], [6, 7]]
    EVENODD = [[0, 2, 4, 6], [1, 3, 5, 7]]

    def bcast(ap_1d, p=P):
        return bass.AP(tensor=ap_1d.tensor, offset=ap_1d.offset,
                       ap=[[0, p]] + list(ap_1d.ap))

    with tile.TileContext(nc) as tc:
        gctx = contextlib.ExitStack()
        with gctx:
            consts = gctx.enter_context(tc.tile_pool(name="consts", bufs=1))
            small = gctx.enter_context(tc.tile_pool(name="small", bufs=4))
            sp = gctx.enter_context(tc.tile_pool(name="sp", bufs=2))
            xp = gctx.enter_context(tc.tile_pool(name="xp", bufs=3))
            xb = gctx.enter_context(tc.tile_pool(name="xb", bufs=2))
            dram = gctx.enter_context(
                tc.tile_pool(name="dram", bufs=1, space="DRAM"))
            ps_a = gctx.enter_context(
                tc.tile_pool(name="ps_a", bufs=4, space="PSUM"))
            ps_av = gctx.enter_context(
                tc.tile_pool(name="ps_av", bufs=2, space="PSUM"))
            ps_tr = gctx.enter_context(
                tc.tile_pool(name="ps_tr", bufs=2, space="PSUM"))

            ident = consts.tile([P, P], bf16)
            make_identity(nc, ident[:])
            eps_t = consts.tile([P, 1], f32)
            nc.vector.memset(eps_t[:], 1e-5)
            mdiag = consts.tile([P, P], f32)
            nc.sync.dma_start(out=mdiag[:], in_=mdiag_d[:])
            mtris = consts.tile([P, P], f32)
            nc.sync.dma_start(out=mtris[:], in_=mtris_d[:])
            mtrii = consts.tile([P, P], f32)
            nc.sync.dma_start(out=mtrii[:], in_=mtrii_d[:])
            cmask_b = consts.tile([P, S], f32)
            nc.sync.dma_start(out=cmask_b[:], in_=bcast(cmask_d[0]))

            ccr_in = [dram.tile([T // 2, D], bf16, name=f"ccr_in{i}")
                      for i in range(2)]
            ccr_out = ([dram.tile([T // 2, D], bf16, name=f"ccr_out{i}")
                        for i in range(2)] if collectives else ccr_in)
            ccf_in = [dram.tile([T // 2, D], bf16, name=f"ccf_in{i}")
                      for i in range(2)]
            ccf_out = ([dram.tile([nb_lm, T // 2, D], bf16,
                                  name=f"ccf_out{i}") for i in range(2)]
                       if collectives else ccf_in)

            def ln_tile(src_ap, dst_ap, s_b, b_b):
                st = small.tile([P, 2, 6], f32, name="lnstats")
                nc.vector.bn_stats(out=st[:, 0, :], in_=src_ap[:, 0:512])
                nc.vector.bn_stats(out=st[:, 1, :], in_=src_ap[:, 512:1024])
                mv = small.tile([P, 2], f32, name="lnmv")
                nc.vector.bn_aggr(out=mv[:], in_=st[:])
                rstd = small.tile([P, 1], f32, name="lnrstd")
                nc.scalar.activation(out=rstd[:], in_=mv[:, 1:2],
                                     func=AF.Sqrt, bias=eps_t[:])
                nc.vector.reciprocal(out=rstd[:], in_=rstd[:])
                tmp = xp.tile([P, D], f32, name="xrow")
                nc.vector.tensor_scalar(out=tmp[:], in0=src_ap,
                                        scalar1=mv[:, 0:1], scalar2=rstd[:],
                                        op0=ALU.subtract, op1=ALU.mult)
                nc.vector.tensor_tensor(out=tmp[:], in0=tmp[:], in1=s_b,
                                        op=ALU.mult)
                nc.vector.tensor_tensor(out=dst_ap, in0=tmp[:], in1=b_b,
                                        op=ALU.add)

            # ---------------- stack phase ----------------
            sctx = contextlib.ExitStack()
            with sctx:
                pers = sctx.enter_context(tc.tile_pool(name="pers", bufs=1))
                wp = sctx.enter_context(tc.tile_pool(name="wp", bufs=2))
                lnp = sctx.enter_context(tc.tile_pool(name="lnp", bufs=1))

                h = pers.tile([P, NT, D], f32)
                encT = pers.tile([P, ND, S], bf16)
                nc.sync.dma_start(
                    out=encT[:],
                    in_=encT_d.rearrange("(k p) s -> p k s", p=P))

                def ln_consts(s_src, b_src):
                    s_b = lnp.tile([P, D], bf16, name="ln_s")
                    nc.gpsimd.dma_start(out=s_b[:], in_=bcast(s_src))
                    b_b = lnp.tile([P, D], bf16, name="ln_b")
                    nc.gpsimd.dma_start(out=b_b[:], in_=bcast(b_src))
                    return s_b, b_b

                def ln_to_xT(dst_xT, s_b, b_b):
                    """x = LN(h) (bf16) then xT[:, k, tt*128:] = T(x)."""
                    for tt in range(NT):
                        xt_ = xb.tile([P, D], bf16, name="xbrow")
                        ln_tile(h[:, tt, :], xt_[:], s_b[:], b_b[:])
                        for k in range(ND):
                            tp_ = ps_tr.tile([P, P], bf16, name="trps")
                            nc.tensor.transpose(
                                tp_[:], xt_[:, k * P:(k + 1) * P], ident[:])
                            if k % 2 == 0:
                                nc.scalar.activation(
                                    out=dst_xT[:, k, tt * P:(tt + 1) * P],
                                    in_=tp_[:], func=AF.Copy)
                            else:
                                nc.vector.tensor_copy(
                                    out=dst_xT[:, k, tt * P:(tt + 1) * P],
                                    in_=tp_[:])

                def proj_qk(dst, w_key, b_sb, scale, src_xT):
                    """dst[pd, m, t] = (xT.T @ W)^T with bias (+opt scale)."""
                    for m in range(MH):
                        wch = wp.tile([P, ND, P], bf16, name="wch")
                        nc.sync.dma_start(out=wch[:], in_=wd[w_key][m])
                        for half in range(2):
                            psq = ps_a.tile([P, 512], f32, name="psq")
                            for k in range(ND):
                                nc.tensor.matmul(
                                    out=psq[:],
                                    lhsT=wch[:, k, :],
                                    rhs=src_xT[:, k,
                                               half * 512:(half + 1) * 512],
                                    start=(k == 0), stop=(k == ND - 1))
                            if scale is None:
                                nc.vector.tensor_scalar(
                                    out=dst[:, m, half * 512:(half + 1) * 512],
                                    in0=psq[:], scalar1=b_sb[:, m:m + 1],
                                    scalar2=None, op0=ALU.add)
                            else:
                                nc.vector.tensor_scalar(
                                    out=dst[:, m, half * 512:(half + 1) * 512],
                                    in0=psq[:], scalar1=b_sb[:, m:m + 1],
                                    scalar2=scale, op0=ALU.add, op1=ALU.mult)

                def ar_h_update(rh, bo_b):
                    """DMA back one token-half of a reduced delta, add bias,
                    accumulate into h."""
                    for tl in range(4):
                        tt = 4 * rh + tl
                        dtile = xb.tile([P, D], bf16, name="xbrow")
                        nc.sync.dma_start(
                            out=dtile[:],
                            in_=ccr_out[rh][tl * P:(tl + 1) * P, :])
                        tmp = xp.tile([P, D], f32, name="xrow")
                        nc.vector.tensor_tensor(out=tmp[:], in0=dtile[:],
                                                in1=bo_b[:], op=ALU.add)
                        nc.vector.tensor_tensor(out=h[:, tt, :],
                                                in0=h[:, tt, :],
                                                in1=tmp[:], op=ALU.add)

                def oproj_ar_update(src_oT, wo_key, bo_key):
                    """o-proj partial -> AllReduce per token half (bf16,
                    pipelined with the next half) -> h update."""
                    bo_b = lnp.tile([P, D], bf16, name="bo_b")
                    nc.gpsimd.dma_start(out=bo_b[:], in_=bcast(bd[bo_key][0]))
                    wchs = []
                    for half in range(2):
                        wch = wp.tile([P, KO, 512], bf16, name="wch")
                        nc.sync.dma_start(out=wch[:], in_=wd[wo_key][half])
                        wchs.append(wch)
                    for rh in range(2):
                        for tl in range(4):
                            tt = 4 * rh + tl
                            for half in range(2):
                                pso = ps_a.tile([P, 512], f32, name="psq")
                                for k in range(KO):
                                    nc.tensor.matmul(
                                        out=pso[:],
                                        lhsT=src_oT[:, k,
                                                    tt * P:(tt + 1) * P],
                                        rhs=wchs[half][:, k, :],
                                        start=(k == 0), stop=(k == KO - 1))
                                ev = xb.tile([P, 512], bf16, name="evb")
                                if (tt + half) % 2 == 0:
                                    nc.scalar.activation(out=ev[:],
                                                         in_=pso[:],
                                                         func=AF.Copy)
                                else:
                                    nc.vector.tensor_copy(out=ev[:],
                                                          in_=pso[:])
                                nc.sync.dma_start(
                                    out=ccr_in[rh][tl * P:(tl + 1) * P,
                                                   half * 512:
                                                   (half + 1) * 512],
                                    in_=ev[:])
                        if collectives:
                            nc.gpsimd.collective_compute(
                                "AllReduce", ALU.add, replica_groups=PAIRS,
                                ins=[ccr_in[rh][:]], outs=[ccr_out[rh][:]])
                        ar_h_update(rh, bo_b)

                # ---- embed + emb LN ----
                lnes, lneb = ln_consts(lnemb_d[0], lnemb_d[1])
                for tt in range(NT):
                    idt = small.tile([P, 1], i32, name="idt")
                    nc.sync.dma_start(out=idt[:],
                                      in_=ids_d[tt * P:(tt + 1) * P])
                    g = xp.tile([P, D], f32, name="xrow")
                    nc.gpsimd.indirect_dma_start(
                        out=g[:], out_offset=None, in_=emb_d[:],
                        in_offset=bass.IndirectOffsetOnAxis(
                            ap=idt[:, :1], axis=0))
                    pt = xp.tile([P, D], f32, name="xrow")
                    nc.sync.dma_start(out=pt[:],
                                      in_=pos_d[tt * P:(tt + 1) * P])
                    nc.vector.tensor_scalar(out=g[:], in0=g[:],
                                            scalar1=EMB_SCALE, scalar2=None,
                                            op0=ALU.mult)
                    nc.vector.tensor_tensor(out=h[:, tt, :], in0=g[:],
                                            in1=pt[:], op=ALU.add)
                    ln_tile(h[:, tt, :], h[:, tt, :], lnes[:], lneb[:])

                for l in range(NL):
                    # ======== self attention ========
                    lns, lnbb = ln_consts(lnw_d[l, 0], lnb_d[l, 0])
                    xT = pers.tile([P, ND, T], bf16, name="xT", tag="xT")
                    ln_to_xT(xT, lns, lnbb)

                    qT = pers.tile([P, MH, T], bf16, name="qT", tag="qT")
                    kT = pers.tile([P, MH, T], bf16, name="kT", tag="kT")
                    vv = pers.tile([P, NT, hsh * HD], bf16, name="vv",
                                   tag="vv")
                    bq_sb = small.tile([P, MH], f32, name="bq")
                    nc.sync.dma_start(out=bq_sb[:], in_=bd[f"bqs{l}"][:])
                    bk_sb = small.tile([P, MH], f32, name="bk")
                    nc.sync.dma_start(out=bk_sb[:], in_=bd[f"bks{l}"][:])
                    bv_b = lnp.tile([P, hsh * HD], bf16, name="bv_b")
                    nc.gpsimd.dma_start(out=bv_b[:],
                                        in_=bcast(bd[f"bvs{l}"][0]))

                    proj_qk(qT, f"wqs{l}", bq_sb, 0.125, xT)
                    proj_qk(kT, f"wks{l}", bk_sb, None, xT)
                    wch_v = wp.tile([P, ND, hsh * HD], bf16, name="wch")
                    nc.sync.dma_start(out=wch_v[:], in_=wd[f"wvs{l}"][0])
                    for tt in range(NT):
                        psv = ps_a.tile([P, 512], f32, name="psq")
                        for k in range(ND):
                            nc.tensor.matmul(
                                out=psv[:],
                                lhsT=xT[:, k, tt * P:(tt + 1) * P],
                                rhs=wch_v[:, k, :],
                                start=(k == 0), stop=(k == ND - 1))
                        nc.vector.tensor_tensor(out=vv[:, tt, :], in0=psv[:],
                                                in1=bv_b[:], op=ALU.add)

                    # scores + softmax + AV per head
                    oT = pers.tile([P, KO, T], bf16, name="oT", tag="oT")
                    for hl in range(hsh):
                        prow = slice((hl % 2) * 64, (hl % 2) * 64 + 64)
                        mq = hl // 2
                        ptile = sp.tile([P, 5, T], bf16, name="ptile")
                        for g in range(NT):
                            width = (g + 2) * P if g < 4 else (g - 3) * P
                            ssb = sp.tile([P, 640], f32, name="srow")
                            qst = qT[prow, mq, g * P:(g + 1) * P]
                            if g < 4:
                                psd = ps_a.tile([P, 512], f32, name="psq")
                                nc.tensor.matmul(
                                    out=psd[:, 0:P], lhsT=qst,
                                    rhs=kT[prow, mq, g * P:(g + 1) * P],
                                    start=True, stop=True)
                                pss = ps_a.tile([P, 512], f32, name="psq")
                                nc.tensor.matmul(
                                    out=pss[:, 0:width - P], lhsT=qst,
                                    rhs=kT[prow, mq, 512:512 + width - P],
                                    start=True, stop=True)
                                nc.vector.tensor_tensor(
                                    out=ssb[:, 0:P], in0=psd[:, 0:P],
                                    in1=mdiag[:], op=ALU.add)
                                if g > 0:
                                    nc.scalar.activation(
                                        out=ssb[:, P:width - P],
                                        in_=pss[:, 0:width - 2 * P],
                                        func=AF.Copy)
                                nc.vector.tensor_tensor(
                                    out=ssb[:, width - P:width],
                                    in0=pss[:, width - 2 * P:width - P],
                                    in1=mtris[:], op=ALU.add)
                            else:
                                pss = ps_a.tile([P, 512], f32, name="psq")
                                nc.tensor.matmul(
                                    out=pss[:, 0:width], lhsT=qst,
                                    rhs=kT[prow, mq, 512:512 + width],
                                    start=True, stop=True)
                                if width > P:
                                    nc.scalar.activation(
                                        out=ssb[:, 0:width - P],
                                        in_=pss[:, 0:width - P], func=AF.Copy)
                                nc.vector.tensor_tensor(
                                    out=ssb[:, width - P:width],
                                    in0=pss[:, width - P:width],
                                    in1=mtrii[:], op=ALU.add)
                            negmax = small.tile([P, 1], f32, name="negmax")
                            nc.vector.tensor_reduce(
                                out=negmax[:], in_=ssb[:, 0:width],
                                axis=AX.X, op=ALU.max, negate=True)
                            probs = sp.tile([P, 640], bf16, name="brow")
                            sums = small.tile([P, 1], f32, name="sums")
                            nc.scalar.activation(
                                out=probs[:, 0:width], in_=ssb[:, 0:width],
                                func=AF.Exp, bias=negmax[:],
                                accum_out=sums[:])
                            recip = small.tile([P, 1], f32, name="recip")
                            nc.vector.reciprocal(out=recip[:], in_=sums[:])
                            nc.vector.tensor_scalar(
                                out=probs[:, 0:width], in0=probs[:, 0:width],
                                scalar1=recip[:], scalar2=None, op0=ALU.mult)
                            if g < 4:
                                chunks = [(4, 0)] + [(mm, (mm + 1) * P)
                                                     for mm in range(g + 1)]
                            else:
                                chunks = [(mm, mm * P)
                                          for mm in range(g - 3)]
                            for ci, (slot, coff) in enumerate(chunks):
                                tpp = ps_tr.tile([P, P], bf16, name="trps")
                                nc.tensor.transpose(
                                    tpp[:], probs[:, coff:coff + P],
                                    ident[:])
                                if (g + ci) % 2 == 0:
                                    nc.scalar.activation(
                                        out=ptile[:, slot,
                                                  g * P:(g + 1) * P],
                                        in_=tpp[:], func=AF.Copy)
                                else:
                                    nc.vector.tensor_copy(
                                        out=ptile[:, slot,
                                                  g * P:(g + 1) * P],
                                        in_=tpp[:])
                        pav_lo = ps_av.tile([64, 512], f32, name="pav")
                        pav_hi = ps_av.tile([64, 512], f32, name="pav")
                        for mm in range(4):
                            nc.tensor.matmul(
                                out=pav_lo[:, mm * P:512],
                                lhsT=vv[:, 4 + mm, hl * HD:(hl + 1) * HD],
                                rhs=ptile[:, mm, mm * P:512],
                                start=(mm == 0), stop=False)
                            nc.tensor.matmul(
                                out=pav_hi[:, mm * P:512],
                                lhsT=vv[:, 4 + mm, hl * HD:(hl + 1) * HD],
                                rhs=ptile[:, mm, 512 + mm * P:T],
                                start=(mm == 0), stop=(mm == 3))
                        for g in range(4):
                            nc.tensor.matmul(
                                out=pav_lo[:, g * P:(g + 1) * P],
                                lhsT=vv[:, g, hl * HD:(hl + 1) * HD],
                                rhs=ptile[:, 4, g * P:(g + 1) * P],
                                start=False, stop=(g == 3))
                        if hl % 2 == 0:
                            nc.scalar.activation(out=oT[prow, mq, 0:512],
                                                 in_=pav_lo[:], func=AF.Copy)
                            nc.vector.tensor_copy(out=oT[prow, mq, 512:T],
                                                  in_=pav_hi[:])
                        else:
                            nc.vector.tensor_copy(out=oT[prow, mq, 0:512],
                                                  in_=pav_lo[:])
                            nc.scalar.activation(out=oT[prow, mq, 512:T],
                                                 in_=pav_hi[:], func=AF.Copy)
                    oproj_ar_update(oT, f"wos{l}", f"bos{l}")

                    # ======== cross attention ========
                    lns2, lnb2 = ln_consts(lnw_d[l, 1], lnb_d[l, 1])
                    x2T = pers.tile([P, ND, T], bf16, name="x2T", tag="xT")
                    ln_to_xT(x2T, lns2, lnb2)
                    q2T = pers.tile([P, MH, T], bf16, name="q2T", tag="qT")
                    bq2_sb = small.tile([P, MH], f32, name="bq")
                    nc.sync.dma_start(out=bq2_sb[:], in_=bd[f"bqc{l}"][:])
                    bk2_sb = small.tile([P, MH], f32, name="bk")
                    nc.sync.dma_start(out=bk2_sb[:], in_=bd[f"bkc{l}"][:])
                    bv2_b = lnp.tile([P, hsh * HD], bf16, name="bv_b")
                    nc.gpsimd.dma_start(out=bv2_b[:],
                                         in_=bcast(bd[f"bvc{l}"][0]))
                    proj_qk(q2T, f"wqc{l}", bq2_sb, 0.125, x2T)

                    wch_k = wp.tile([P, ND, MH, P], bf16, name="wch")
                    for m in range(MH):
                        nc.sync.dma_start(out=wch_k[:, :, m, :],
                                          in_=wd[f"wkc{l}"][m])
                    kenc_raw = sp.tile([S, hsh * HD], bf16, name="kenc", bufs=1)
                    psk = ps_a.tile([P, 512], f32, name="psq")
                    for k in range(ND):
                        nc.tensor.matmul(out=psk[:], lhsT=encT[:, k, :],
                                         rhs=wch_k[:, k, :, :],
                                         start=(k == 0), stop=(k == ND - 1))
                    nc.scalar.activation(out=kenc_raw[:], in_=psk[:],
                                         func=AF.Copy)
                    kTe = sp.tile([P, MH, S], bf16, name="kTe", bufs=1)
                    for m in range(MH):
                        tpk = ps_tr.tile([P, P], bf16, name="trps")
                        nc.tensor.transpose(
                            tpk[:], kenc_raw[:, m * P:(m + 1) * P], ident[:])
                        nc.vector.tensor_scalar(
                            out=kTe[:, m, :], in0=tpk[:],
                            scalar1=bk2_sb[:, m:m + 1], scalar2=None,
                            op0=ALU.add)
                    wch_v2 = wp.tile([P, ND, hsh * HD], bf16, name="wch")
                    nc.sync.dma_start(out=wch_v2[:], in_=wd[f"wvc{l}"][0])
                    venc = sp.tile([S, hsh * HD], bf16, name="venc", bufs=1)
                    psv2 = ps_a.tile([P, 512], f32, name="psq")
                    for k in range(ND):
                        nc.tensor.matmul(out=psv2[:], lhsT=encT[:, k, :],
                                         rhs=wch_v2[:, k, :],
                                         start=(k == 0), stop=(k == ND - 1))
                    nc.vector.tensor_tensor(out=venc[:], in0=psv2[:],
                                            in1=bv2_b[:], op=ALU.add)

                    o2T = pers.tile([P, KO, T], bf16, name="o2T", tag="oT")
                    for hl in range(hsh):
                        prow = slice((hl % 2) * 64, (hl % 2) * 64 + 64)
                        mq = hl // 2
                        p2tile = sp.tile([S, T], bf16, name="ptile")
                        for tt in range(NT):
                            ps2 = ps_a.tile([P, 512], f32, name="psq")
                            nc.tensor.matmul(
                                out=ps2[:, 0:S],
                                lhsT=q2T[prow, mq, tt * P:(tt + 1) * P],
                                rhs=kTe[prow, mq, :], start=True, stop=True)
                            s2 = sp.tile([P, S], f32, name="srow")
                            nc.vector.tensor_tensor(out=s2[:],
                                                    in0=ps2[:, 0:S],
                                                    in1=cmask_b[:],
                                                    op=ALU.add)
                            negmax = small.tile([P, 1], f32, name="negmax")
                            nc.vector.tensor_reduce(
                                out=negmax[:], in_=s2[:], axis=AX.X,
                                op=ALU.max, negate=True)
                            probs2 = sp.tile([P, S], bf16, name="brow")
                            sums = small.tile([P, 1], f32, name="sums")
                            nc.scalar.activation(
                                out=probs2[:], in_=s2[:], func=AF.Exp,
                                bias=negmax[:], accum_out=sums[:])
                            recip = small.tile([P, 1], f32, name="recip")
                            nc.vector.reciprocal(out=recip[:], in_=sums[:])
                            nc.vector.tensor_scalar(
                                out=probs2[:], in0=probs2[:],
                                scalar1=recip[:], scalar2=None, op0=ALU.mult)
                            tpp = ps_tr.tile([P, P], bf16, name="trps")
                            nc.tensor.transpose(tpp[:], probs2[:], ident[:])
                            nc.scalar.activation(
                                out=p2tile[:, tt * P:(tt + 1) * P],
                                in_=tpp[:], func=AF.Copy)
                        for half in range(2):
                            pav2 = ps_av.tile([64, 512], f32, name="pav")
                            nc.tensor.matmul(
                                out=pav2[:],
                                lhsT=venc[:, hl * HD:(hl + 1) * HD],
                                rhs=p2tile[:, half * 512:(half + 1) * 512],
                                start=True, stop=True)
                            nc.scalar.activation(
                                out=o2T[prow, mq, half * 512:(half + 1) * 512],
                                in_=pav2[:], func=AF.Copy)
                    oproj_ar_update(o2T, f"woc{l}", f"boc{l}")

                    # ======== FFN ========
                    NDT = dsh // P
                    lns3, lnb3 = ln_consts(lnw_d[l, 2], lnb_d[l, 2])
                    x3T = pers.tile([P, ND, T], bf16, name="x3T", tag="xT")
                    ln_to_xT(x3T, lns3, lnb3)
                    bf1_sb = small.tile([P, NDT], f32, name="bf1s")
                    nc.sync.dma_start(out=bf1_sb[:], in_=bd[f"bf1{l}"][:])
                    bf2_b = lnp.tile([P, D], bf16, name="bo_b")
                    nc.gpsimd.dma_start(out=bf2_b[:],
                                         in_=bcast(bd[f"bf2{l}"][0]))
                    # fc1: stationary weights -> g^T directly, fused
                    # bias+gelu on eviction
                    gT = pers.tile([P, NDT, T], bf16, name="gT", tag="oT")
                    for dfft in range(NDT):
                        f1c = wp.tile([P, ND, P], bf16, name="wch")
                        nc.sync.dma_start(out=f1c[:], in_=wd[f"wf1{l}"][dfft])
                        for th in range(2):
                            psf = ps_a.tile([P, 512], f32, name="psq")
                            for k in range(ND):
                                nc.tensor.matmul(
                                    out=psf[:], lhsT=f1c[:, k, :],
                                    rhs=x3T[:, k, th * 512:(th + 1) * 512],
                                    start=(k == 0), stop=(k == ND - 1))
                            nc.scalar.activation(
                                out=gT[:, dfft, th * 512:(th + 1) * 512],
                                in_=psf[:], func=gelu,
                                bias=bf1_sb[:, dfft:dfft + 1])
                    # fc2: accumulate over dff tiles in PSUM per (tt, n2)
                    f2cs = []
                    for n2 in range(2):
                        f2c = wp.tile([P, NDT, 512], bf16, name="wch")
                        nc.sync.dma_start(out=f2c[:], in_=wd[f"wf2{l}"][n2])
                        f2cs.append(f2c)
                    for rh in range(2):
                        for tl in range(4):
                            tt = 4 * rh + tl
                            for n2 in range(2):
                                psf2 = ps_a.tile([P, 512], f32, name="psq")
                                for kk in range(NDT):
                                    nc.tensor.matmul(
                                        out=psf2[:],
                                        lhsT=gT[:, kk, tt * P:(tt + 1) * P],
                                        rhs=f2cs[n2][:, kk, :],
                                        start=(kk == 0),
                                        stop=(kk == NDT - 1))
                                ev = xb.tile([P, 512], bf16, name="evb")
                                if (tt + n2) % 2 == 0:
                                    nc.scalar.activation(out=ev[:],
                                                         in_=psf2[:],
                                                         func=AF.Copy)
                                else:
                                    nc.vector.tensor_copy(out=ev[:],
                                                          in_=psf2[:])
                                nc.sync.dma_start(
                                    out=ccr_in[rh][tl * P:(tl + 1) * P,
                                                   n2 * 512:(n2 + 1) * 512],
                                    in_=ev[:])
                        if collectives:
                            nc.gpsimd.collective_compute(
                                "AllReduce", ALU.add, replica_groups=PAIRS,
                                ins=[ccr_in[rh][:]], outs=[ccr_out[rh][:]])
                        ar_h_update(rh, bf2_b)

                # ---- final LN -> ccf ----
                lnfs, lnfb = ln_consts(lnfin_d[0], lnfin_d[1])
                for rh in range(2):
                    for tl in range(4):
                        hf = xb.tile([P, D], bf16, name="xbrow")
                        ln_tile(h[:, 4 * rh + tl, :], hf[:], lnfs[:],
                                lnfb[:])
                        nc.sync.dma_start(
                            out=ccf_in[rh][tl * P:(tl + 1) * P, :],
                            in_=hf[:])
                    if collectives:
                        nc.gpsimd.collective_compute(
                            "AllGather", ALU.bypass, replica_groups=EVENODD,
                            ins=[ccf_in[rh][:]], outs=[ccf_out[rh][:]])
            # stack pools closed here

            # ---------------- LM head ----------------
            lctx = contextlib.ExitStack()
            with lctx:
                lmp = lctx.enter_context(tc.tile_pool(name="lmp", bufs=1))
                lmt = lctx.enter_context(tc.tile_pool(name="lmt", bufs=2))
                wlm_sb = lmp.tile([P, ND, vsh], bf16)
                for k in range(ND):
                    nc.sync.dma_start(out=wlm_sb[:, k, :], in_=wlm_d[:, k, :])
                for bt in range(nb_lm):
                    for tt in range(NT):
                        hft = xb.tile([P, D], bf16, name="xbrow")
                        rh, tl = tt // 4, tt % 4
                        src = (ccf_out[rh][bt, tl * P:(tl + 1) * P, :]
                               if collectives
                               else ccf_in[rh][tl * P:(tl + 1) * P, :])
                        nc.sync.dma_start(out=hft[:], in_=src)
                        hfT = lmt.tile([P, ND, P], bf16, name="hfT")
                        for k in range(ND):
                            tph = ps_tr.tile([P, P], bf16, name="trps")
                            nc.tensor.transpose(
                                tph[:], hft[:, k * P:(k + 1) * P], ident[:])
                            if k % 2 == 0:
                                nc.scalar.activation(out=hfT[:, k, :],
                                                     in_=tph[:],
                                                     func=AF.Copy)
                            else:
                                nc.vector.tensor_copy(out=hfT[:, k, :],
                                                      in_=tph[:])
                        for v in range(vsh // 512):
                            psl = ps_a.tile([P, 512], f32, name="psq")
                            for k in range(ND):
                                nc.tensor.matmul(
                                    out=psl[:], lhsT=hfT[:, k, :],
                                    rhs=wlm_sb[:, k, v * 512:(v + 1) * 512],
                                    start=(k == 0), stop=(k == ND - 1))
                            osb = xp.tile([P, 512], f32, name="ev512")
                            if v % 2 == 0:
                                nc.scalar.activation(out=osb[:], in_=psl[:],
                                                     func=AF.Copy)
                            else:
                                nc.vector.tensor_copy(out=osb[:], in_=psl[:])
                            nc.sync.dma_start(
                                out=out_d[(bt * NT + tt) * P:
                                          (bt * NT + tt + 1) * P,
                                          v * 512:(v + 1) * 512],
                                in_=osb[:])
    nc.compile()
    return nc


_NC_CACHE = {}


def _get_nc(key):
    if key not in _NC_CACHE:
        hsh, dsh, vsh, nb_lm, coll = key
        _NC_CACHE[key] = build_nc(hsh, dsh, vsh, nb_lm, coll)
    return _NC_CACHE[key]


def kernel(**inputs) -> np.ndarray:
    nc = _get_nc((8, 2048, VSH, 4, True))
    maps = host_prepare(inputs, hsh=8, dsh=2048, vsh=VSH)
    res = run_bass_kernel_spmd(nc, maps, core_ids=list(range(8)),
                               trace=False)
    logits = np.concatenate([res.results[c]["out"] for c in range(8)], axis=1)
    return np.ascontiguousarray(
        logits[:, : V + 1].reshape(B, T, V + 1).astype(np.float32))


# revision 14
# speedup vs baseline: 1.3211x; 1.0338x over previous
"""BlockDiffusionDecoder (mBART-style 2-layer decoder + BD3LM self-attn mask)
on 8 Trainium2 NeuronCores.

Sharding: cores (2b, 2b+1) own batch element b (B=4 -> 8 cores).  Within a
pair, tensor-parallel over heads (8 of 16) and d_ff (2048 of 4096), with a
pair AllReduce after the o-projections and fc2.  The LM head is sharded over
vocab 8 ways (padded 32768 = 8 x 4096) after an AllGather of final hidden
states across the {even} / {odd} core groups.

Layouts: activations live in SBUF as [128 tokens, tile, feature]; transposed
copies ([feature-tile, token]) are built with PE transposes.  All matmuls run
in bf16 (full PE rate); residual stream / LN / softmax stats stay fp32.
Weights are shipped from host pre-tiled and pre-cast to bf16.
"""
import sys

if "/opt/trn_rl_repo" not in sys.path:
    sys.path.insert(0, "/opt/trn_rl_repo")

import contextlib

import ml_dtypes
import numpy as np

import concourse.bass as bass
import concourse.bacc as bacc
import concourse.tile as tile
from concourse import mybir
from concourse.bass_utils import run_bass_kernel_spmd
from concourse.masks import make_identity

P = 128
B, D, H, NL, DFF, V, S = 4, 1024, 16, 2, 4096, 32000, 128
T = 1024
HD = D // H          # 64
BLK = 4
VP = 32768           # padded vocab (32001 -> 8*4096)
VSH = VP // 8        # vocab shard per core
NT = T // P          # 8 token tiles
ND = D // P          # 8 feature tiles
EMB_SCALE = 32.0     # sqrt(D)
FMIN = float(np.finfo(np.float32).min)
BF = ml_dtypes.bfloat16

f32 = mybir.dt.float32
bf16 = mybir.dt.bfloat16
i32 = mybir.dt.int32
AF = mybir.ActivationFunctionType
ALU = mybir.AluOpType
AX = mybir.AxisListType


def _rhs_tile(w_t: np.ndarray, nchunk: int) -> np.ndarray:
    """[d_in, d_out] -> [n_chunks, 128, k_tiles, nchunk] bf16, so the DMA of
    one n-chunk is contiguous per partition (k-major, n-minor)."""
    d_in, d_out = w_t.shape
    kt = d_in // P
    nc_ = d_out // nchunk
    return np.ascontiguousarray(
        w_t.reshape(kt, P, nc_, nchunk).transpose(2, 1, 0, 3).astype(BF))


def host_prepare(inputs: dict, hsh: int, dsh: int, vsh: int):
    """Build per-core input maps. hsh: heads/core, dsh: d_ff/core."""
    tp = 16 // hsh
    ids = np.asarray(inputs["input_ids"])
    enc = np.asarray(inputs["enc_hidden"], dtype=np.float32)
    emask = np.asarray(inputs["enc_mask"])
    emb = np.ascontiguousarray(np.asarray(inputs["embed_tokens"], np.float32))
    pos = np.ascontiguousarray(np.asarray(inputs["pos_embed"], np.float32))
    attn_w = np.asarray(inputs["attn_w"], np.float32)
    attn_b = np.asarray(inputs["attn_b"], np.float32)
    ln_w = np.asarray(inputs["ln_w"], np.float32)
    ln_b = np.asarray(inputs["ln_b"], np.float32)
    fc1_w = np.asarray(inputs["fc1_w"], np.float32)
    fc1_b = np.asarray(inputs["fc1_b"], np.float32)
    fc2_w = np.asarray(inputs["fc2_w"], np.float32)
    fc2_b = np.asarray(inputs["fc2_b"], np.float32)
    lm_w = np.asarray(inputs["lm_head_w"], np.float32)

    lm_pad = np.zeros((VP, D), np.float32)
    lm_pad[: V + 1] = lm_w
    lm_t = lm_pad.T  # [D, VP]

    n_cores = 8 if tp == 2 else 1
    maps = []
    for c in range(n_cores):
        b_ = c // tp
        j = c % tp
        hs = slice(j * hsh * HD, (j + 1) * hsh * HD)
        ds_ = slice(j * dsh, (j + 1) * dsh)
        vs_ = slice(c * vsh, (c + 1) * vsh) if tp == 2 else slice(0, vsh)
        m = {
            "ids": ids[b_].reshape(T, 1).astype(np.int32),
            "emb": emb,
            "pos": pos,
            "encT": np.ascontiguousarray(enc[b_].T.astype(BF)),   # [D, S]
            "cmask": ((1.0 - emask[b_].astype(np.float32)) * FMIN)
            .reshape(1, S),
            "lnemb": np.stack([np.asarray(inputs["ln_emb_s"], np.float32),
                               np.asarray(inputs["ln_emb_b"], np.float32)]),
            "lnfin": np.stack([np.asarray(inputs["final_ln_s"], np.float32),
                               np.asarray(inputs["final_ln_b"], np.float32)]),
            "lnw": ln_w, "lnb": ln_b,
            "wlm": np.ascontiguousarray(
                lm_t[:, vs_].reshape(ND, P, vsh).transpose(1, 0, 2)
                .astype(BF)),
        }
        for l in range(NL):
            for a, tag in ((0, "s"), (1, "c")):
                wq, wk, wv, wo = attn_w[l, a]
                bq, bk, bv, bo = attn_b[l, a]
                m[f"wq{tag}{l}"] = _rhs_tile(wq.T[:, hs], P)
                m[f"wk{tag}{l}"] = _rhs_tile(wk.T[:, hs], P)
                m[f"wv{tag}{l}"] = _rhs_tile(wv.T[:, hs], hsh * HD)
                m[f"wo{tag}{l}"] = _rhs_tile(wo.T[hs, :], D // 2)
                mh = hsh * HD // P
                m[f"bq{tag}{l}"] = np.ascontiguousarray(
                    bq[hs].reshape(mh, P).T)
                m[f"bk{tag}{l}"] = np.ascontiguousarray(
                    bk[hs].reshape(mh, P).T)
                m[f"bv{tag}{l}"] = bv[hs].reshape(1, hsh * HD).copy()
                m[f"bo{tag}{l}"] = (bo / tp).reshape(1, D).copy()
            m[f"wf1{l}"] = _rhs_tile(fc1_w[l].T[:, ds_], P)
            m[f"bf1{l}"] = np.ascontiguousarray(
                fc1_b[l][ds_].reshape(dsh // P, P).T)
            m[f"wf2{l}"] = np.ascontiguousarray(
                fc2_w[l].T[ds_, :].reshape(dsh // P, P, 2, 512)
                .transpose(2, 1, 0, 3).astype(BF))  # [n2, p, kk, 512]
            m[f"bf2{l}"] = (fc2_b[l] / tp).reshape(1, D).copy()
        maps.append(m)
    return maps


def _mask_consts():
    i = np.arange(P)
    diag = np.where((i[:, None] // BLK) == (i[None, :] // BLK), 0.0, FMIN)
    tri_s = np.where((i[:, None] // BLK) > (i[None, :] // BLK), 0.0, FMIN)
    tri_i = np.where((i[:, None] // BLK) >= (i[None, :] // BLK), 0.0, FMIN)
    return (diag.astype(np.float32), tri_s.astype(np.float32),
            tri_i.astype(np.float32))


def build_nc(hsh=8, dsh=2048, vsh=VSH, nb_lm=4, collectives=True,
             gelu=AF.Gelu_apprx_tanh):
    tp = 16 // hsh
    MH = hsh * HD // P        # d_out tiles for q/k/v shard
    KO = MH                   # k-tiles for o-proj lhs
    NDC = dsh // 512          # dff chunks
    nc = bacc.Bacc(num_devices=8 if collectives else None, trn_type="TRN2")

    ids_d = nc.dram_tensor("ids", [T, 1], i32, kind="ExternalInput")
    emb_d = nc.dram_tensor("emb", [V + 1, D], f32, kind="ExternalInput")
    pos_d = nc.dram_tensor("pos", [T, D], f32, kind="ExternalInput")
    encT_d = nc.dram_tensor("encT", [D, S], bf16, kind="ExternalInput")
    cmask_d = nc.dram_tensor("cmask", [1, S], f32, kind="ExternalInput")
    lnemb_d = nc.dram_tensor("lnemb", [2, D], f32, kind="ExternalInput")
    lnfin_d = nc.dram_tensor("lnfin", [2, D], f32, kind="ExternalInput")
    lnw_d = nc.dram_tensor("lnw", [NL, 3, D], f32, kind="ExternalInput")
    lnb_d = nc.dram_tensor("lnb", [NL, 3, D], f32, kind="ExternalInput")
    wlm_d = nc.dram_tensor("wlm", [P, ND, vsh], bf16, kind="ExternalInput")
    wd, bd = {}, {}
    for l in range(NL):
        for tg in ("s", "c"):
            wd[f"wq{tg}{l}"] = nc.dram_tensor(
                f"wq{tg}{l}", [MH, P, ND, P], bf16, kind="ExternalInput")
            wd[f"wk{tg}{l}"] = nc.dram_tensor(
                f"wk{tg}{l}", [MH, P, ND, P], bf16, kind="ExternalInput")
            wd[f"wv{tg}{l}"] = nc.dram_tensor(
                f"wv{tg}{l}", [1, P, ND, hsh * HD], bf16,
                kind="ExternalInput")
            wd[f"wo{tg}{l}"] = nc.dram_tensor(
                f"wo{tg}{l}", [2, P, KO, D // 2], bf16, kind="ExternalInput")
            bd[f"bq{tg}{l}"] = nc.dram_tensor(
                f"bq{tg}{l}", [P, MH], f32, kind="ExternalInput")
            bd[f"bk{tg}{l}"] = nc.dram_tensor(
                f"bk{tg}{l}", [P, MH], f32, kind="ExternalInput")
            bd[f"bv{tg}{l}"] = nc.dram_tensor(
                f"bv{tg}{l}", [1, hsh * HD], f32, kind="ExternalInput")
            bd[f"bo{tg}{l}"] = nc.dram_tensor(
                f"bo{tg}{l}", [1, D], f32, kind="ExternalInput")
        wd[f"wf1{l}"] = nc.dram_tensor(
            f"wf1{l}", [dsh // P, P, ND, P], bf16, kind="ExternalInput")
        bd[f"bf1{l}"] = nc.dram_tensor(
            f"bf1{l}", [P, dsh // P], f32, kind="ExternalInput")
        wd[f"wf2{l}"] = nc.dram_tensor(
            f"wf2{l}", [2, P, dsh // P, 512], bf16, kind="ExternalInput")
        bd[f"bf2{l}"] = nc.dram_tensor(
            f"bf2{l}", [1, D], f32, kind="ExternalInput")
    out_d = nc.dram_tensor("out", [nb_lm * T, vsh], f32,
                           kind="ExternalOutput")

    mdiag_np, mtris_np, mtrii_np = _mask_consts()
    mdiag_d = nc.inline_tensor(mdiag_np, "mdiag")
    mtris_d = nc.inline_tensor(mtris_np, "mtris")
    mtrii_d = nc.inline_tensor(mtrii_np, "mtrii")

    PAIRS = [[0, 1], [2, 3], [4, 5], [6, 7]]
    EVENODD = [[0, 2, 4, 6], [1, 3, 5, 7]]

    def bcast(ap_1d, p=P):
        return bass.AP(tensor=ap_1d.tensor, offset=ap_1d.offset,
                       ap=[[0, p]] + list(ap_1d.ap))

    with tile.TileContext(nc) as tc:
        gctx = contextlib.ExitStack()
        with gctx:
            consts = gctx.enter_context(tc.tile_pool(name="consts", bufs=1))
            small = gctx.enter_context(tc.tile_pool(name="small", bufs=4))
            sp = gctx.enter_context(tc.tile_pool(name="sp", bufs=2))
            xp = gctx.enter_context(tc.tile_pool(name="xp", bufs=3))
            xb = gctx.enter_context(tc.tile_pool(name="xb", bufs=2))
            dram = gctx.enter_context(
                tc.tile_pool(name="dram", bufs=1, space="DRAM"))
            ps_a = gctx.enter_context(
                tc.tile_pool(name="ps_a", bufs=4, space="PSUM"))
            ps_av = gctx.enter_context(
                tc.tile_pool(name="ps_av", bufs=2, space="PSUM"))
            ps_tr = gctx.enter_context(
                tc.tile_pool(name="ps_tr", bufs=2, space="PSUM"))

            ident = consts.tile([P, P], bf16)
            make_identity(nc, ident[:])
            eps_t = consts.tile([P, 1], f32)
            nc.vector.memset(eps_t[:], 1e-5)
            mdiag = consts.tile([P, P], f32)
            nc.sync.dma_start(out=mdiag[:], in_=mdiag_d[:])
            mtris = consts.tile([P, P], f32)
            nc.sync.dma_start(out=mtris[:], in_=mtris_d[:])
            mtrii = consts.tile([P, P], f32)
            nc.sync.dma_start(out=mtrii[:], in_=mtrii_d[:])
            cmask_b = consts.tile([P, S], f32)
            nc.sync.dma_start(out=cmask_b[:], in_=bcast(cmask_d[0]))

            ccr_in = [dram.tile([T // 2, D], bf16, name=f"ccr_in{i}")
                      for i in range(2)]
            ccr_out = ([dram.tile([T // 2, D], bf16, name=f"ccr_out{i}")
                        for i in range(2)] if collectives else ccr_in)
            ccf_in = [dram.tile([T // 2, D], bf16, name=f"ccf_in{i}")
                      for i in range(2)]
            ccf_out = ([dram.tile([nb_lm, T // 2, D], bf16,
                                  name=f"ccf_out{i}") for i in range(2)]
                       if collectives else ccf_in)

            def ln_tile(src_ap, dst_ap, s_b, b_b):
                st = small.tile([P, 2, 6], f32, name="lnstats")
                nc.vector.bn_stats(out=st[:, 0, :], in_=src_ap[:, 0:512])
                nc.vector.bn_stats(out=st[:, 1, :], in_=src_ap[:, 512:1024])
                mv = small.tile([P, 2], f32, name="lnmv")
                nc.vector.bn_aggr(out=mv[:], in_=st[:])
                rstd = small.tile([P, 1], f32, name="lnrstd")
                nc.scalar.activation(out=rstd[:], in_=mv[:, 1:2],
                                     func=AF.Sqrt, bias=eps_t[:])
                nc.vector.reciprocal(out=rstd[:], in_=rstd[:])
                tmp = xp.tile([P, D], f32, name="xrow")
                nc.vector.tensor_scalar(out=tmp[:], in0=src_ap,
                                        scalar1=mv[:, 0:1], scalar2=rstd[:],
                                        op0=ALU.subtract, op1=ALU.mult)
                nc.vector.tensor_tensor(out=tmp[:], in0=tmp[:], in1=s_b,
                                        op=ALU.mult)
                nc.vector.tensor_tensor(out=dst_ap, in0=tmp[:], in1=b_b,
                                        op=ALU.add)

            # ---------------- stack phase ----------------
            sctx = contextlib.ExitStack()
            with sctx:
                pers = sctx.enter_context(tc.tile_pool(name="pers", bufs=1))
                wp = sctx.enter_context(tc.tile_pool(name="wp", bufs=2))
                lnp = sctx.enter_context(tc.tile_pool(name="lnp", bufs=1))

                h = pers.tile([P, NT, D], f32)
                encT = pers.tile([P, ND, S], bf16)
                nc.sync.dma_start(
                    out=encT[:],
                    in_=encT_d.rearrange("(k p) s -> p k s", p=P))

                def ln_consts(s_src, b_src):
                    s_b = lnp.tile([P, D], bf16, name="ln_s")
                    nc.gpsimd.dma_start(out=s_b[:], in_=bcast(s_src))
                    b_b = lnp.tile([P, D], bf16, name="ln_b")
                    nc.gpsimd.dma_start(out=b_b[:], in_=bcast(b_src))
                    return s_b, b_b

                def ln_to_xT(dst_xT, s_b, b_b):
                    """x = LN(h) (bf16) then xT[:, k, tt*128:] = T(x)."""
                    for tt in range(NT):
                        xt_ = xb.tile([P, D], bf16, name="xbrow")
                        ln_tile(h[:, tt, :], xt_[:], s_b[:], b_b[:])
                        for k in range(ND):
                            tp_ = ps_tr.tile([P, P], bf16, name="trps")
                            nc.tensor.transpose(
                                tp_[:], xt_[:, k * P:(k + 1) * P], ident[:])
                            if k % 2 == 0:
                                nc.scalar.activation(
                                    out=dst_xT[:, k, tt * P:(tt + 1) * P],
                                    in_=tp_[:], func=AF.Copy)
                            else:
                                nc.vector.tensor_copy(
                                    out=dst_xT[:, k, tt * P:(tt + 1) * P],
                                    in_=tp_[:])

                def proj_qk(dst, w_key, b_sb, scale, src_xT):
                    """dst[pd, m, t] = (xT.T @ W)^T with bias (+opt scale)."""
                    for m in range(MH):
                        wch = wp.tile([P, ND, P], bf16, name="wch")
                        nc.sync.dma_start(out=wch[:], in_=wd[w_key][m])
                        for half in range(2):
                            psq = ps_a.tile([P, 512], f32, name="psq")
                            for k in range(ND):
                                nc.tensor.matmul(
                                    out=psq[:],
                                    lhsT=wch[:, k, :],
                                    rhs=src_xT[:, k,
                                               half * 512:(half + 1) * 512],
                                    start=(k == 0), stop=(k == ND - 1))
                            if scale is None:
                                nc.vector.tensor_scalar(
                                    out=dst[:, m, half * 512:(half + 1) * 512],
                                    in0=psq[:], scalar1=b_sb[:, m:m + 1],
                                    scalar2=None, op0=ALU.add)
                            else:
                                nc.vector.tensor_scalar(
                                    out=dst[:, m, half * 512:(half + 1) * 512],
                                    in0=psq[:], scalar1=b_sb[:, m:m + 1],
                                    scalar2=scale, op0=ALU.add, op1=ALU.mult)

                def ar_h_update(rh):
                    """DMA back one token-half of a reduced delta and
                    accumulate into h (bias was folded pre-AllReduce)."""
                    for tl in range(4):
                        tt = 4 * rh + tl
                        dtile = xb.tile([P, D], bf16, name="xbrow")
                        nc.sync.dma_start(
                            out=dtile[:],
                            in_=ccr_out[rh][tl * P:(tl + 1) * P, :])
                        nc.vector.tensor_tensor(out=h[:, tt, :],
                                                in0=h[:, tt, :],
                                                in1=dtile[:], op=ALU.add)

                def oproj_ar_update(src_oT, wo_key, bo_key):
                    """o-proj partial -> AllReduce per token half (bf16,
                    pipelined with the next half) -> h update."""
                    bo_b = lnp.tile([P, D], bf16, name="bo_b")
                    nc.gpsimd.dma_start(out=bo_b[:], in_=bcast(bd[bo_key][0]))
                    wchs = []
                    for half in range(2):
                        wch = wp.tile([P, KO, 512], bf16, name="wch")
                        nc.sync.dma_start(out=wch[:], in_=wd[wo_key][half])
                        wchs.append(wch)
                    for rh in range(2):
                        for tl in range(4):
                            tt = 4 * rh + tl
                            for half in range(2):
                                pso = ps_a.tile([P, 512], f32, name="psq")
                                for k in range(KO):
                                    nc.tensor.matmul(
                                        out=pso[:],
                                        lhsT=src_oT[:, k,
                                                    tt * P:(tt + 1) * P],
                                        rhs=wchs[half][:, k, :],
                                        start=(k == 0), stop=(k == KO - 1))
                                ev = xb.tile([P, 512], bf16, name="evb")
                                nc.vector.tensor_tensor(
                                    out=ev[:], in0=pso[:],
                                    in1=bo_b[:, half * 512:(half + 1) * 512],
                                    op=ALU.add)
                                nc.sync.dma_start(
                                    out=ccr_in[rh][tl * P:(tl + 1) * P,
                                                   half * 512:
                                                   (half + 1) * 512],
                                    in_=ev[:])
                        if collectives:
                            nc.gpsimd.collective_compute(
                                "AllReduce", ALU.add, replica_groups=PAIRS,
                                ins=[ccr_in[rh][:]], outs=[ccr_out[rh][:]])
                        ar_h_update(rh)

                # ---- embed + emb LN ----
                lnes, lneb = ln_consts(lnemb_d[0], lnemb_d[1])
                for tt in range(NT):
                    idt = small.tile([P, 1], i32, name="idt")
                    nc.sync.dma_start(out=idt[:],
                                      in_=ids_d[tt * P:(tt + 1) * P])
                    g = xp.tile([P, D], f32, name="xrow")
                    nc.gpsimd.indirect_dma_start(
                        out=g[:], out_offset=None, in_=emb_d[:],
                        in_offset=bass.IndirectOffsetOnAxis(
                            ap=idt[:, :1], axis=0))
                    pt = xp.tile([P, D], f32, name="xrow")
                    nc.sync.dma_start(out=pt[:],
                                      in_=pos_d[tt * P:(tt + 1) * P])
                    nc.vector.tensor_scalar(out=g[:], in0=g[:],
                                            scalar1=EMB_SCALE, scalar2=None,
                                            op0=ALU.mult)
                    nc.vector.tensor_tensor(out=h[:, tt, :], in0=g[:],
                                            in1=pt[:], op=ALU.add)
                    ln_tile(h[:, tt, :], h[:, tt, :], lnes[:], lneb[:])

                for l in range(NL):
                    # ======== self attention ========
                    lns, lnbb = ln_consts(lnw_d[l, 0], lnb_d[l, 0])
                    xT = pers.tile([P, ND, T], bf16, name="xT", tag="xT")
                    ln_to_xT(xT, lns, lnbb)

                    qT = pers.tile([P, MH, T], bf16, name="qT", tag="qT")
                    kT = pers.tile([P, MH, T], bf16, name="kT", tag="kT")
                    vv = pers.tile([P, NT, hsh * HD], bf16, name="vv",
                                   tag="vv")
                    bq_sb = small.tile([P, MH], f32, name="bq")
                    nc.sync.dma_start(out=bq_sb[:], in_=bd[f"bqs{l}"][:])
                    bk_sb = small.tile([P, MH], f32, name="bk")
                    nc.sync.dma_start(out=bk_sb[:], in_=bd[f"bks{l}"][:])
                    bv_b = lnp.tile([P, hsh * HD], bf16, name="bv_b")
                    nc.gpsimd.dma_start(out=bv_b[:],
                                        in_=bcast(bd[f"bvs{l}"][0]))

                    proj_qk(qT, f"wqs{l}", bq_sb, 0.125, xT)
                    proj_qk(kT, f"wks{l}", bk_sb, None, xT)
                    wch_v = wp.tile([P, ND, hsh * HD], bf16, name="wch")
                    nc.sync.dma_start(out=wch_v[:], in_=wd[f"wvs{l}"][0])
                    for tt in range(NT):
                        psv = ps_a.tile([P, 512], f32, name="psq")
                        for k in range(ND):
                            nc.tensor.matmul(
                                out=psv[:],
                                lhsT=xT[:, k, tt * P:(tt + 1) * P],
                                rhs=wch_v[:, k, :],
                                start=(k == 0), stop=(k == ND - 1))
                        nc.vector.tensor_tensor(out=vv[:, tt, :], in0=psv[:],
                                                in1=bv_b[:], op=ALU.add)

                    # scores + softmax + AV per head
                    oT = pers.tile([P, KO, T], bf16, name="oT", tag="oT")
                    for hl in range(hsh):
                        prow = slice((hl % 2) * 64, (hl % 2) * 64 + 64)
                        mq = hl // 2
                        ptile = sp.tile([P, 5, T], bf16, name="ptile")
                        for g in range(NT):
                            width = (g + 2) * P if g < 4 else (g - 3) * P
                            ssb = sp.tile([P, 640], f32, name="srow")
                            qst = qT[prow, mq, g * P:(g + 1) * P]
                            if g < 4:
                                psd = ps_a.tile([P, 512], f32, name="psq")
                                nc.tensor.matmul(
                                    out=psd[:, 0:P], lhsT=qst,
                                    rhs=kT[prow, mq, g * P:(g + 1) * P],
                                    start=True, stop=True)
                                pss = ps_a.tile([P, 512], f32, name="psq")
                                nc.tensor.matmul(
                                    out=pss[:, 0:width - P], lhsT=qst,
                                    rhs=kT[prow, mq, 512:512 + width - P],
                                    start=True, stop=True)
                                nc.vector.tensor_tensor(
                                    out=ssb[:, 0:P], in0=psd[:, 0:P],
                                    in1=mdiag[:], op=ALU.add)
                                if g > 0:
                                    nc.scalar.activation(
                                        out=ssb[:, P:width - P],
                                        in_=pss[:, 0:width - 2 * P],
                                        func=AF.Copy)
                                nc.vector.tensor_tensor(
                                    out=ssb[:, width - P:width],
                                    in0=pss[:, width - 2 * P:width - P],
                                    in1=mtris[:], op=ALU.add)
                            else:
                                pss = ps_a.tile([P, 512], f32, name="psq")
                                nc.tensor.matmul(
                                    out=pss[:, 0:width], lhsT=qst,
                                    rhs=kT[prow, mq, 512:512 + width],
                                    start=True, stop=True)
                                if width > P:
                                    nc.scalar.activation(
                                        out=ssb[:, 0:width - P],
                                        in_=pss[:, 0:width - P], func=AF.Copy)
                                nc.vector.tensor_tensor(
                                    out=ssb[:, width - P:width],
                                    in0=pss[:, width - P:width],
                                    in1=mtrii[:], op=ALU.add)
                            negmax = small.tile([P, 1], f32, name="negmax")
                            nc.vector.tensor_reduce(
                                out=negmax[:], in_=ssb[:, 0:width],
                                axis=AX.X, op=ALU.max, negate=True)
                            probs = sp.tile([P, 640], bf16, name="brow")
                            sums = small.tile([P, 1], f32, name="sums")
                            nc.scalar.activation(
                                out=probs[:, 0:width], in_=ssb[:, 0:width],
                                func=AF.Exp, bias=negmax[:],
                                accum_out=sums[:])
                            recip = small.tile([P, 1], f32, name="recip")
                            nc.vector.reciprocal(out=recip[:], in_=sums[:])
                            nc.vector.tensor_scalar(
                                out=probs[:, 0:width], in0=probs[:, 0:width],
                                scalar1=recip[:], scalar2=None, op0=ALU.mult)
                            if g < 4:
                                chunks = [(4, 0)] + [(mm, (mm + 1) * P)
                                                     for mm in range(g + 1)]
                            else:
                                chunks = [(mm, mm * P)
                                          for mm in range(g - 3)]
                            for ci, (slot, coff) in enumerate(chunks):
                                tpp = ps_tr.tile([P, P], bf16, name="trps")
                                nc.tensor.transpose(
                                    tpp[:], probs[:, coff:coff + P],
                                    ident[:])
                                if (g + ci) % 2 == 0:
                                    nc.scalar.activation(
                                        out=ptile[:, slot,
                                                  g * P:(g + 1) * P],
                                        in_=tpp[:], func=AF.Copy)
                                else:
                                    nc.vector.tensor_copy(
                                        out=ptile[:, slot,
                                                  g * P:(g + 1) * P],
                                        in_=tpp[:])
                        pav_lo = ps_av.tile([64, 512], f32, name="pav")
                        pav_hi = ps_av.tile([64, 512], f32, name="pav")
                        for mm in range(4):
                            nc.tensor.matmul(
                                out=pav_lo[:, mm * P:512],
                                lhsT=vv[:, 4 + mm, hl * HD:(hl + 1) * HD],
                                rhs=ptile[:, mm, mm * P:512],
                                start=(mm == 0), stop=False)
                            nc.tensor.matmul(
                                out=pav_hi[:, mm * P:512],
                                lhsT=vv[:, 4 + mm, hl * HD:(hl + 1) * HD],
                                rhs=ptile[:, mm, 512 + mm * P:T],
                                start=(mm == 0), stop=(mm == 3))
                        for g in range(4):
                            nc.tensor.matmul(
                                out=pav_lo[:, g * P:(g + 1) * P],
                                lhsT=vv[:, g, hl * HD:(hl + 1) * HD],
                                rhs=ptile[:, 4, g * P:(g + 1) * P],
                                start=False, stop=(g == 3))
                        if hl % 2 == 0:
                            nc.scalar.activation(out=oT[prow, mq, 0:512],
                                                 in_=pav_lo[:], func=AF.Copy)
                            nc.vector.tensor_copy(out=oT[prow, mq, 512:T],
                                                  in_=pav_hi[:])
                        else:
                            nc.vector.tensor_copy(out=oT[prow, mq, 0:512],
                                                  in_=pav_lo[:])
                            nc.scalar.activation(out=oT[prow, mq, 512:T],
                                                 in_=pav_hi[:], func=AF.Copy)
                    oproj_ar_update(oT, f"wos{l}", f"bos{l}")

                    # ======== cross attention ========
                    lns2, lnb2 = ln_consts(lnw_d[l, 1], lnb_d[l, 1])
                    x2T = pers.tile([P, ND, T], bf16, name="x2T", tag="xT")
                    ln_to_xT(x2T, lns2, lnb2)
                    q2T = pers.tile([P, MH, T], bf16, name="q2T", tag="qT")
                    bq2_sb = small.tile([P, MH], f32, name="bq")
                    nc.sync.dma_start(out=bq2_sb[:], in_=bd[f"bqc{l}"][:])
                    bk2_sb = small.tile([P, MH], f32, name="bk")
                    nc.sync.dma_start(out=bk2_sb[:], in_=bd[f"bkc{l}"][:])
                    bv2_b = lnp.tile([P, hsh * HD], bf16, name="bv_b")
                    nc.gpsimd.dma_start(out=bv2_b[:],
                                         in_=bcast(bd[f"bvc{l}"][0]))
                    proj_qk(q2T, f"wqc{l}", bq2_sb, 0.125, x2T)

                    wch_k = wp.tile([P, ND, MH, P], bf16, name="wch")
                    for m in range(MH):
                        nc.sync.dma_start(out=wch_k[:, :, m, :],
                                          in_=wd[f"wkc{l}"][m])
                    kenc_raw = sp.tile([S, hsh * HD], bf16, name="kenc", bufs=1)
                    psk = ps_a.tile([P, 512], f32, name="psq")
                    for k in range(ND):
                        nc.tensor.matmul(out=psk[:], lhsT=encT[:, k, :],
                                         rhs=wch_k[:, k, :, :],
                                         start=(k == 0), stop=(k == ND - 1))
                    nc.scalar.activation(out=kenc_raw[:], in_=psk[:],
                                         func=AF.Copy)
                    kTe = sp.tile([P, MH, S], bf16, name="kTe", bufs=1)
                    for m in range(MH):
                        tpk = ps_tr.tile([P, P], bf16, name="trps")
                        nc.tensor.transpose(
                            tpk[:], kenc_raw[:, m * P:(m + 1) * P], ident[:])
                        nc.vector.tensor_scalar(
                            out=kTe[:, m, :], in0=tpk[:],
                            scalar1=bk2_sb[:, m:m + 1], scalar2=None,
                            op0=ALU.add)
                    wch_v2 = wp.tile([P, ND, hsh * HD], bf16, name="wch")
                    nc.sync.dma_start(out=wch_v2[:], in_=wd[f"wvc{l}"][0])
                    venc = sp.tile([S, hsh * HD], bf16, name="venc", bufs=1)
                    psv2 = ps_a.tile([P, 512], f32, name="psq")
                    for k in range(ND):
                        nc.tensor.matmul(out=psv2[:], lhsT=encT[:, k, :],
                                         rhs=wch_v2[:, k, :],
                                         start=(k == 0), stop=(k == ND - 1))
                    nc.vector.tensor_tensor(out=venc[:], in0=psv2[:],
                                            in1=bv2_b[:], op=ALU.add)

                    o2T = pers.tile([P, KO, T], bf16, name="o2T", tag="oT")
                    for hl in range(hsh):
                        prow = slice((hl % 2) * 64, (hl % 2) * 64 + 64)
                        mq = hl // 2
                        p2tile = sp.tile([S, T], bf16, name="ptile")
                        for tt in range(NT):
                            ps2 = ps_a.tile([P, 512], f32, name="psq")
                            nc.tensor.matmul(
                                out=ps2[:, 0:S],
                                lhsT=q2T[prow, mq, tt * P:(tt + 1) * P],
                                rhs=kTe[prow, mq, :], start=True, stop=True)
                            s2 = sp.tile([P, S], f32, name="srow")
                            nc.vector.tensor_tensor(out=s2[:],
                                                    in0=ps2[:, 0:S],
                                                    in1=cmask_b[:],
                                                    op=ALU.add)
                            negmax = small.tile([P, 1], f32, name="negmax")
                            nc.vector.tensor_reduce(
                                out=negmax[:], in_=s2[:], axis=AX.X,
                                op=ALU.max, negate=True)
                            probs2 = sp.tile([P, S], bf16, name="brow")
                            sums = small.tile([P, 1], f32, name="sums")
                            nc.scalar.activation(
                                out=probs2[:], in_=s2[:], func=AF.Exp,
                                bias=negmax[:], accum_out=sums[:])
                            recip = small.tile([P, 1], f32, name="recip")
                            nc.vector.reciprocal(out=recip[:], in_=sums[:])
                            nc.vector.tensor_scalar(
                                out=probs2[:], in0=probs2[:],
                                scalar1=recip[:], scalar2=None, op0=ALU.mult)
                            tpp = ps_tr.tile([P, P], bf16, name="trps")
                            nc.tensor.transpose(tpp[:], probs2[:], ident[:])
                            nc.scalar.activation(
                                out=p2tile[:, tt * P:(tt + 1) * P],
                                in_=tpp[:], func=AF.Copy)
                        for half in range(2):
                            pav2 = ps_av.tile([64, 512], f32, name="pav")
                            nc.tensor.matmul(
                                out=pav2[:],
                                lhsT=venc[:, hl * HD:(hl + 1) * HD],
                                rhs=p2tile[:, half * 512:(half + 1) * 512],
                                start=True, stop=True)
                            nc.scalar.activation(
                                out=o2T[prow, mq, half * 512:(half + 1) * 512],
                                in_=pav2[:], func=AF.Copy)
                    oproj_ar_update(o2T, f"woc{l}", f"boc{l}")

                    # ======== FFN ========
                    NDT = dsh // P
                    lns3, lnb3 = ln_consts(lnw_d[l, 2], lnb_d[l, 2])
                    x3T = pers.tile([P, ND, T], bf16, name="x3T", tag="xT")
                    ln_to_xT(x3T, lns3, lnb3)
                    bf1_sb = small.tile([P, NDT], f32, name="bf1s")
                    nc.sync.dma_start(out=bf1_sb[:], in_=bd[f"bf1{l}"][:])
                    bf2_b = lnp.tile([P, D], bf16, name="bo_b")
                    nc.gpsimd.dma_start(out=bf2_b[:],
                                         in_=bcast(bd[f"bf2{l}"][0]))
                    # fc1: stationary weights -> g^T directly, fused
                    # bias+gelu on eviction
                    gT = pers.tile([P, NDT, T], bf16, name="gT", tag="oT")
                    for dfft in range(NDT):
                        f1c = wp.tile([P, ND, P], bf16, name="wch")
                        nc.sync.dma_start(out=f1c[:], in_=wd[f"wf1{l}"][dfft])
                        for th in range(2):
                            psf = ps_a.tile([P, 512], f32, name="psq")
                            for k in range(ND):
                                nc.tensor.matmul(
                                    out=psf[:], lhsT=f1c[:, k, :],
                                    rhs=x3T[:, k, th * 512:(th + 1) * 512],
                                    start=(k == 0), stop=(k == ND - 1))
                            nc.scalar.activation(
                                out=gT[:, dfft, th * 512:(th + 1) * 512],
                                in_=psf[:], func=gelu,
                                bias=bf1_sb[:, dfft:dfft + 1])
                    # fc2: accumulate over dff tiles in PSUM per (tt, n2)
                    f2cs = []
                    for n2 in range(2):
                        f2c = wp.tile([P, NDT, 512], bf16, name="wch")
                        nc.sync.dma_start(out=f2c[:], in_=wd[f"wf2{l}"][n2])
                        f2cs.append(f2c)
                    for rh in range(2):
                        for tl in range(4):
                            tt = 4 * rh + tl
                            for n2 in range(2):
                                psf2 = ps_a.tile([P, 512], f32, name="psq")
                                for kk in range(NDT):
                                    nc.tensor.matmul(
                                        out=psf2[:],
                                        lhsT=gT[:, kk, tt * P:(tt + 1) * P],
                                        rhs=f2cs[n2][:, kk, :],
                                        start=(kk == 0),
                                        stop=(kk == NDT - 1))
                                ev = xb.tile([P, 512], bf16, name="evb")
                                nc.vector.tensor_tensor(
                                    out=ev[:], in0=psf2[:],
                                    in1=bf2_b[:, n2 * 512:(n2 + 1) * 512],
                                    op=ALU.add)
                                nc.sync.dma_start(
                                    out=ccr_in[rh][tl * P:(tl + 1) * P,
                                                   n2 * 512:(n2 + 1) * 512],
                                    in_=ev[:])
                        if collectives:
                            nc.gpsimd.collective_compute(
                                "AllReduce", ALU.add, replica_groups=PAIRS,
                                ins=[ccr_in[rh][:]], outs=[ccr_out[rh][:]])
                        ar_h_update(rh)

                # ---- final LN -> ccf ----
                lnfs, lnfb = ln_consts(lnfin_d[0], lnfin_d[1])
                for rh in range(2):
                    for tl in range(4):
                        hf = xb.tile([P, D], bf16, name="xbrow")
                        ln_tile(h[:, 4 * rh + tl, :], hf[:], lnfs[:],
                                lnfb[:])
                        nc.sync.dma_start(
                            out=ccf_in[rh][tl * P:(tl + 1) * P, :],
                            in_=hf[:])
                    if collectives:
                        nc.gpsimd.collective_compute(
                            "AllGather", ALU.bypass, replica_groups=EVENODD,
                            ins=[ccf_in[rh][:]], outs=[ccf_out[rh][:]])
            # stack pools closed here

            # ---------------- LM head ----------------
            lctx = contextlib.ExitStack()
            with lctx:
                lmp = lctx.enter_context(tc.tile_pool(name="lmp", bufs=1))
                lmt = lctx.enter_context(tc.tile_pool(name="lmt", bufs=2))
                wlm_sb = lmp.tile([P, ND, vsh], bf16)
                for k in range(ND):
                    nc.sync.dma_start(out=wlm_sb[:, k, :], in_=wlm_d[:, k, :])
                for bt in range(nb_lm):
                    for tt in range(NT):
                        hft = xb.tile([P, D], bf16, name="xbrow")
                        rh, tl = tt // 4, tt % 4
                        src = (ccf_out[rh][bt, tl * P:(tl + 1) * P, :]
                               if collectives
                               else ccf_in[rh][tl * P:(tl + 1) * P, :])
                        nc.sync.dma_start(out=hft[:], in_=src)
                        hfT = lmt.tile([P, ND, P], bf16, name="hfT")
                        for k in range(ND):
                            tph = ps_tr.tile([P, P], bf16, name="trps")
                            nc.tensor.transpose(
                                tph[:], hft[:, k * P:(k + 1) * P], ident[:])
                            if k % 2 == 0:
                                nc.scalar.activation(out=hfT[:, k, :],
                                                     in_=tph[:],
                                                     func=AF.Copy)
                            else:
                                nc.vector.tensor_copy(out=hfT[:, k, :],
                                                      in_=tph[:])
                        for v in range(vsh // 512):
                            psl = ps_a.tile([P, 512], f32, name="psq")
                            for k in range(ND):
                                nc.tensor.matmul(
                                    out=psl[:], lhsT=hfT[:, k, :],
                                    rhs=wlm_sb[:, k, v * 512:(v + 1) * 512],
                                    start=(k == 0), stop=(k == ND - 1))
                            osb = xp.tile([P, 512], f32, name="ev512")
                            if v % 2 == 0:
                                nc.scalar.activation(out=osb[:], in_=psl[:],
                                                     func=AF.Copy)
                            else:
                                nc.vector.tensor_copy(out=osb[:], in_=psl[:])
                            nc.sync.dma_start(
                                out=out_d[(bt * NT + tt) * P:
                                          (bt * NT + tt + 1) * P,
                                          v * 512:(v + 1) * 512],
                                in_=osb[:])
    nc.compile()
    return nc


_NC_CACHE = {}


def _get_nc(key):
    if key not in _NC_CACHE:
        hsh, dsh, vsh, nb_lm, coll = key
        _NC_CACHE[key] = build_nc(hsh, dsh, vsh, nb_lm, coll)
    return _NC_CACHE[key]


def kernel(**inputs) -> np.ndarray:
    nc = _get_nc((8, 2048, VSH, 4, True))
    maps = host_prepare(inputs, hsh=8, dsh=2048, vsh=VSH)
    res = run_bass_kernel_spmd(nc, maps, core_ids=list(range(8)),
                               trace=False)
    logits = np.concatenate([res.results[c]["out"] for c in range(8)], axis=1)
    return np.ascontiguousarray(
        logits[:, : V + 1].reshape(B, T, V + 1).astype(np.float32))
